# revision 6
# baseline (speedup 1.0000x reference)
"""LoRA QKV fused projection kernel for 8 TRN2 NeuronCores.

Reference computation (T=8192 tokens, HID=4096, D=6144 out, S=8 slots, R=16):
    y = x @ W.T
    a[t,s,i,r] = sum_h x[t,h] * lora_A[s,i,r,h]         (down-proj, all slots)
    a *= onehot(token_to_slot)[t,s] * scaling[s]         (routing gate)
    d[t, :] = concat_i( sum_{s,r} a[t,s,i,r] * B_i[s,:,r] )   (up-proj)
    out = y + d
Sharding: data-parallel over tokens; core c owns tokens [c*1024, (c+1)*1024).

Per-core dataflow (v2):
  * Phase A (LoRA down-proj aT = A @ x) in fp8 e4m3 DoubleRow over all 16
    k-tile PAIRS, j-major into 6 psum chains (3 targets x 2 token halves).
    The packed pax table ([A targets | x tokens] per (j,pair) row) streams
    in j-chunks on the scalar queue, pacing the A matmuls.
  * mb0's main k-chain is interleaved BETWEEN Phase A j-groups: its bf16
    k-tiles are paced by the x_res chunks arriving on the sync queue, so
    the PE stays fed from ~8us while both streams warm up (banks: 6 A +
    2 for mb0 = 8 exactly; mb0's B-matmul lands right after the gate).
  * Phase B: routing gate (host-built onehot*scaling, expanded over rank)
    applied on DVE: ag = psum_a * gate, written bf16.
  * Phase C per output row-block mb (48): 24 bf16 k-tiles (W pre-scaled by
    SP=2^15 to match the fp8 psum scale) + 4 fp8 DoubleRow k-tile PAIRS
    (reusing the pax x8 slices) accumulate into 2 psum tiles (token
    halves); the LoRA up-proj B[mb] @ ag[i] accumulates INTO THE SAME
    psum (start=False), fusing base + delta. psum->sbuf copy descales by
    1/SP exactly, then one DMA out per half (last 2 mb split into 256-col
    pieces to shorten the tail).

Precision (gate 2e-2): bf16 main + fp8 on 8/32 k-tiles + fp8 LoRA delta
measured 1.921e-2 on the harness inputs (numpy sim matches HW to ~1e-6).
fp8 e4m3 DoubleRow measured ~234ns per 512-col pair vs 2x218ns bf16.
"""

import numpy as np
import ml_dtypes

# problem shape (hardcoded per harness contract)
T = 8192
HID = 4096
Q_SIZE = 4096
KV_SIZE = 1024
D = Q_SIZE + 2 * KV_SIZE  # 6144
S = 8
R = 16
NCORES = 8
P = 128

TC = T // NCORES          # 1024 tokens per core
MB = D // P               # 48 output row-blocks of 128
KA = HID // P             # 32 k-tiles
NP8 = 4                   # k-tile PAIRS of the main GEMM done in fp8 DoubleRow
KB = KA - 2 * NP8         # 24 k-tiles of the main GEMM done in bf16
NH = TC // 512            # 2 token halves (psum bank = 512 fp32)
JA = KA // 2              # 16 k-tile pairs
SP = np.float32(32768.0)  # psum scale SX*SA: bf16 W is pre-scaled by SP so
                          # fp8 (x*32)(W*1024) products accumulate coherently;
                          # the final psum->sbuf copy descales by 1/SP (exact)

_CACHE = {}


def _build_nc():
    import concourse.mybir as mybir
    import concourse.tile as tile
    from concourse import bacc

    bf16 = mybir.dt.bfloat16
    f32 = mybir.dt.float32
    f8 = mybir.dt.float8e4
    DR = mybir.MatmulPerfMode.DoubleRow

    nc = bacc.Bacc(None, target_bir_lowering=False, debug=False)

    # ---- DRAM parameters (per-core shapes)
    PAX = 3 * P + TC  # per-(j,pair) packed row: 3 A targets then x tokens
    x_d = nc.declare_dram_parameter("x_sh", [P, KB, TC], bf16, isOutput=False)
    w_d = nc.declare_dram_parameter("w_t", [MB, P, KB, P], bf16, isOutput=False)
    w8_d = nc.declare_dram_parameter("w8_t", [MB, P, NP8, 2, P], f8, isOutput=False)
    pax_d = nc.declare_dram_parameter("pax", [P, JA, 2, PAX], f8, isOutput=False)
    b_d = nc.declare_dram_parameter("b_t", [P, MB, P], bf16, isOutput=False)
    g_d = nc.declare_dram_parameter("gate", [P, TC], f32, isOutput=False)
    y_d = nc.declare_dram_parameter("y_out", [MB, P, TC], f32, isOutput=True)

    with tile.TileContext(nc) as tc:
        with tc.tile_pool(name="xres", bufs=1) as xres_pool, \
             tc.tile_pool(name="wp", bufs=3) as w_pool, \
             tc.tile_pool(name="ab", bufs=1) as ab_pool, \
             tc.tile_pool(name="agp", bufs=1) as ag_pool, \
             tc.tile_pool(name="stp", bufs=3) as st_pool, \
             tc.tile_pool(name="psum", bufs=8, space="PSUM") as ps_pool:

            # resident operands
            x_res = xres_pool.tile([P, KB, TC], bf16, tag="xres")
            pax_t = ab_pool.tile([P, JA, 2, PAX], f8, tag="pax")
            b_t = ab_pool.tile([P, MB, P], bf16, tag="b")
            gate_t = ab_pool.tile([P, TC], f32, tag="gate")

            # Warmup is DMA-bound: ~12.5MB (w0, x, pax, gate) must land before
            # the mb pipeline is self-sustaining. Spread it over THREE rings:
            #   sync:   x k0-15 (4 chunks), b[8:], even-mb outputs
            #   scalar: w0 (split), x k16-23, w1, then the W stream, odd outs
            #   gpsimd: pax j-chunks (pacing Phase A), gate, b[0:8]
            for (lo, hi) in [(0, 2), (2, 6), (6, 11), (11, 16)]:
                nc.sync.dma_start(out=x_res[:, lo:hi, :], in_=x_d[:, lo:hi, :])
            nc.sync.dma_start(out=b_t[:, 8:MB], in_=b_d[:, 8:MB])

            def load_w(mb, queue):
                w_t = w_pool.tile([P, KB, P], bf16, tag="w", name=f"w{mb}")
                queue.dma_start(out=w_t[:], in_=w_d[mb])
                w8_t = w_pool.tile([P, NP8, 2, P], f8, tag="w8", name=f"w8{mb}")
                queue.dma_start(out=w8_t[:], in_=w8_d[mb])
                return w_t, w8_t

            # w0 split so mb0's first matmuls start ~3us in
            w0_t = w_pool.tile([P, KB, P], bf16, tag="w", name="w0")
            nc.scalar.dma_start(out=w0_t[:, 0:6, :], in_=w_d[0, :, 0:6, :])
            nc.scalar.dma_start(out=w0_t[:, 6:KB, :], in_=w_d[0, :, 6:KB, :])
            w80_t = w_pool.tile([P, NP8, 2, P], f8, tag="w8", name="w80")
            nc.scalar.dma_start(out=w80_t[:], in_=w8_d[0])
            nc.scalar.dma_start(out=x_res[:, 16:20, :], in_=x_d[:, 16:20, :])
            w1_t, w81_t = load_w(1, nc.scalar)
            nc.scalar.dma_start(out=x_res[:, 20:KB, :], in_=x_d[:, 20:KB, :])

            jch = [1, 1, 2, 4, 4, 4]  # pax j-tiles per streamed chunk
            j0 = 0
            for jc in jch:
                nc.gpsimd.dma_start(
                    out=pax_t[:, j0:j0 + jc], in_=pax_d[:, j0:j0 + jc])
                j0 += jc
            nc.gpsimd.dma_start(out=gate_t[:], in_=g_d[:])
            nc.gpsimd.dma_start(out=b_t[:, 0:8], in_=b_d[:, 0:8])

            # ---------------- PE emission ------------------------------------
            # Phase A psums (6 banks) + mb0's 2 psums = 8 banks.
            ps_a = [
                ps_pool.tile([P, 512], f32, tag="ps", name=f"ps_a{i}_{h}")
                for i in range(3) for h in range(NH)
            ]
            ps0 = [ps_pool.tile([P, 512], f32, tag="ps", name=f"pm0_{h}")
                   for h in range(NH)]

            def a_jgroup(jlo, jhi):
                for j in range(jlo, jhi):
                    for i in range(3):
                        for h in range(NH):
                            nc.tensor.matmul(
                                ps_a[i * NH + h][:],
                                pax_t[:, j, :, i * P:(i + 1) * P],
                                pax_t[:, j, :, 3 * P + h * 512:3 * P + (h + 1) * 512],
                                start=(j == 0), stop=(j == JA - 1),
                                perf_mode=DR,
                            )

            def mb0_ksub(klo, khi):
                for k in range(klo, khi):
                    for h in range(NH):
                        nc.tensor.matmul(
                            ps0[h][:],
                            w0_t[:, k, :],
                            x_res[:, k, h * 512:(h + 1) * 512],
                            start=(k == 0), stop=False,
                        )

            # Interleave: A j-groups (paced by pax on vector) with mb0
            # k-subchains (paced by x_res on sync/scalar), roughly matching
            # the DMA arrival schedule of each stream.
            mb0_ksub(0, 2)
            a_jgroup(0, 1)
            mb0_ksub(2, 4)
            a_jgroup(1, 2)
            mb0_ksub(4, 6)
            a_jgroup(2, 4)
            mb0_ksub(6, 10)
            a_jgroup(4, 8)
            mb0_ksub(10, 14)
            a_jgroup(8, 12)
            mb0_ksub(14, 18)
            a_jgroup(12, JA)
            mb0_ksub(18, KB)
            # mb0 fp8 pairs (pax j=12..15 now resident)
            for jp in range(NP8):
                j = KB // 2 + jp
                for h in range(NH):
                    nc.tensor.matmul(
                        ps0[h][:],
                        w80_t[:, jp, :, :],
                        pax_t[:, j, :, 3 * P + h * 512:3 * P + (h + 1) * 512],
                        start=False, stop=False,
                        perf_mode=DR,
                    )

            # ---------------- Phase B: routing gate ------------------------
            ag = []
            for i in range(3):
                ag_t = ag_pool.tile([P, TC], bf16, tag=f"ag{i}", name=f"ag{i}")
                for h in range(NH):
                    sl = slice(h * 512, (h + 1) * 512)
                    nc.vector.tensor_mul(ag_t[:, sl], ps_a[i * NH + h][:], gate_t[:, sl])
                ag.append(ag_t)

            # ---------------- mb0 B-matmul + copy-out ----------------------
            def finish_mb(mb, pss):
                i = 0 if mb < Q_SIZE // P else (1 if mb < (Q_SIZE + KV_SIZE) // P else 2)
                st = st_pool.tile([P, TC], f32, tag="st", name=f"st{mb}")
                # alternate output queues; split the last mbs' DMAs to
                # shorten the tail
                oq = nc.sync if mb % 2 == 0 else nc.scalar
                pieces = 2 if mb >= MB - 2 else 1
                for h in range(NH):
                    nc.tensor.matmul(
                        pss[h][:],
                        b_t[:, mb, :],
                        ag[i][:, h * 512:(h + 1) * 512],
                        start=False, stop=True,
                    )
                    nc.vector.tensor_scalar_mul(
                        st[:, h * 512:(h + 1) * 512], pss[h][:],
                        float(1.0 / SP))
                    pw = 512 // pieces
                    for pc in range(pieces):
                        lo = h * 512 + pc * pw
                        oq.dma_start(
                            out=y_d[mb, :, lo:lo + pw],
                            in_=st[:, lo:lo + pw],
                        )

            finish_mb(0, ps0)

            # ------------- Phase C: remaining mb chains ---------------------
            for mb in range(1, MB):
                w_t, w8_t = (w1_t, w81_t) if mb == 1 else load_w(mb, nc.scalar)
                pss = [
                    ps_pool.tile([P, 512], f32, tag="ps", name=f"pm{mb}_{h}")
                    for h in range(NH)
                ]
                for k in range(KB):
                    for h in range(NH):
                        nc.tensor.matmul(
                            pss[h][:],
                            w_t[:, k, :],
                            x_res[:, k, h * 512:(h + 1) * 512],
                            start=(k == 0), stop=False,
                        )
                for jp in range(NP8):
                    j = KB // 2 + jp
                    for h in range(NH):
                        nc.tensor.matmul(
                            pss[h][:],
                            w8_t[:, jp, :, :],
                            pax_t[:, j, :, 3 * P + h * 512:3 * P + (h + 1) * 512],
                            start=False, stop=False,
                            perf_mode=DR,
                        )
                finish_mb(mb, pss)

    nc.compile()
    return nc


def _get_nc():
    if "nc" not in _CACHE:
        _CACHE["nc"] = _build_nc()
    return _CACHE["nc"]


def _prep_in_maps(x, W, lora_A, lora_B_q, lora_B_k, lora_B_v, scaling, token_to_slot):
    f = np.float32
    bf = ml_dtypes.bfloat16
    x = np.ascontiguousarray(x, dtype=f)
    W = np.ascontiguousarray(W, dtype=f)

    # x shard, moving operand: [c, p, ka, tl]  (h = ka*128 + p, t = c*1024 + tl)
    x_f32 = np.ascontiguousarray(
        x.reshape(NCORES, TC, KA, P).transpose(0, 3, 2, 1))
    x_sh = np.ascontiguousarray(x_f32[:, :, :KB, :]).astype(bf)
    # W stationary: [mb, p, ka, dl]  (d = mb*128 + dl)  -- replicated.
    # bf16 part pre-scaled by SP to match the fp8 psum scale; the last
    # 2*NP8 k-tiles go as fp8(W*1024) DoubleRow pairs.
    w_all = W.reshape(MB, P, KA, P).transpose(0, 3, 2, 1)
    w_t = np.ascontiguousarray(w_all[:, :, :KB, :] * SP).astype(bf)
    # fp8 e4m3 copies for the LoRA down-proj (DoubleRow pairs of k-tiles),
    # packed [A targets | x tokens] per (j, pair) row so each chunk is one
    # DMA; the 1/(SX*SA) descale folds into the gate below.
    SX, SA = np.float32(32.0), np.float32(1024.0)
    f8 = ml_dtypes.float8_e4m3
    w8 = np.ascontiguousarray(
        (w_all[:, :, KB:, :] * SA).astype(f8).reshape(MB, P, NP8, 2, P))
    x8 = (x_f32 * SX).astype(f8).reshape(NCORES, P, JA, 2, TC)
    a_f32 = np.ascontiguousarray(
        np.asarray(lora_A, dtype=f).reshape(S, 3, R, KA, P).transpose(4, 3, 1, 0, 2)
        .reshape(P, KA, 3, S * R))
    a8 = (a_f32 * SA).astype(f8).reshape(P, JA, 2, 3 * S * R)
    pax = np.concatenate(
        [np.broadcast_to(a8, (NCORES,) + a8.shape), x8], axis=-1)
    pax = np.ascontiguousarray(pax)
    # LoRA B stationary: [(s r), mb, dl] -- replicated
    bq = np.asarray(lora_B_q, dtype=f).transpose(0, 2, 1).reshape(S * R, Q_SIZE)
    bk = np.asarray(lora_B_k, dtype=f).transpose(0, 2, 1).reshape(S * R, KV_SIZE)
    bv = np.asarray(lora_B_v, dtype=f).transpose(0, 2, 1).reshape(S * R, KV_SIZE)
    b_t = np.ascontiguousarray(
        np.concatenate([bq, bk, bv], axis=1).reshape(S * R, MB, P)).astype(bf)
    # routing gate, expanded over ranks: [c, (s r), tl]. The LoRA psum is
    # already SP x true scale (x*32 times A*1024), which matches the main
    # psum scale, so the gate is just the per-slot scaling.
    slot = np.asarray(token_to_slot).reshape(NCORES, TC)
    g = (slot[:, None, :] == np.arange(S, dtype=slot.dtype)[None, :, None])
    g = g.astype(f) * np.asarray(scaling, dtype=f)[None, :, None]
    gate = np.ascontiguousarray(np.repeat(g, R, axis=1))

    in_maps = []
    for c in range(NCORES):
        in_maps.append({
            "x_sh": x_sh[c],
            "w_t": w_t,
            "w8_t": w8,
            "pax": pax[c],
            "b_t": b_t,
            "gate": gate[c],
        })
    return in_maps


def _assemble(results):
    out = np.empty((T, D), dtype=np.float32)
    for c in range(NCORES):
        out[c * TC:(c + 1) * TC, :] = results[c]["y_out"].reshape(D, TC).T
    return out


def _run(inputs, trace=False):
    from concourse.bass_utils import run_bass_kernel_spmd
    nc = _get_nc()
    in_maps = _prep_in_maps(**inputs)
    res = run_bass_kernel_spmd(
        nc, in_maps, core_ids=list(range(NCORES)), trace=trace)
    return res


def kernel(**inputs) -> np.ndarray:
    res = _run(inputs, trace=False)
    return _assemble(res.results)


if __name__ == "__main__":
    rng = np.random.default_rng(0)
    ins = {
        "x": rng.standard_normal((T, HID)).astype(np.float32),
        "W": (rng.standard_normal((D, HID)) * 0.02).astype(np.float32),
        "lora_A": (rng.standard_normal((S, 3, R, HID)) * 0.02).astype(np.float32),
        "lora_B_q": (rng.standard_normal((S, Q_SIZE, R)) * 0.02).astype(np.float32),
        "lora_B_k": (rng.standard_normal((S, KV_SIZE, R)) * 0.02).astype(np.float32),
        "lora_B_v": (rng.standard_normal((S, KV_SIZE, R)) * 0.02).astype(np.float32),
        "scaling": rng.uniform(0.5, 2.0, S).astype(np.float32),
        "token_to_slot": rng.integers(0, S, T).astype(np.int32),
    }
    out = kernel(**ins)
    print("out", out.shape, out.dtype)


# revision 9
# speedup vs baseline: 1.0198x; 1.0198x over previous
"""LoRA QKV fused projection kernel for 8 TRN2 NeuronCores.

Reference computation (T=8192 tokens, HID=4096, D=6144 out, S=8 slots, R=16):
    y = x @ W.T
    a[t,s,i,r] = sum_h x[t,h] * lora_A[s,i,r,h]         (down-proj, all slots)
    a *= onehot(token_to_slot)[t,s] * scaling[s]         (routing gate)
    d[t, :] = concat_i( sum_{s,r} a[t,s,i,r] * B_i[s,:,r] )   (up-proj)
    out = y + d
Sharding: data-parallel over tokens; core c owns tokens [c*1024, (c+1)*1024).

Per-core dataflow (v5):
  * Phase A (LoRA down-proj aT = A @ x) in fp8 e4m3 DoubleRow over all 16
    k-tile PAIRS, one token half at a time (3 psum banks), j-major, paced
    by the pax tables streaming on the scalar ring. pax is split into
    pax1 = [A targets | x8 half0] and pax2 = [x8 half1] so half0 can
    start early.
  * mb0 and mb1's bf16 k-chains interleave BETWEEN Phase A j-groups
    (banks: 3 A + 2+2 = 7 of 8); their fp8 pairs + B-matmuls run right
    after the gate. PE stays fed from ~5us while ~14MB of warmup DMA
    (w0/w1, x, pax, gate) streams on two rings (gpsimd as a third ring
    measured WORSE: its DRAIN ops add ~14us of idle).
  * Phase B: routing gate (host-built onehot*scaling, expanded over rank)
    applied on DVE per half: ag = psum_a * gate, written bf16.
  * Phase C per output row-block mb (48): 24 bf16 k-tiles (W pre-scaled by
    SP=2^15 to match the fp8 psum scale) + 4 fp8 DoubleRow k-tile PAIRS
    accumulate into 2 psum tiles (token halves); the LoRA up-proj
    B[mb] @ ag[i] accumulates INTO THE SAME psum (start=False), fusing
    base + delta. psum->sbuf copy descales by 1/SP exactly, then one DMA
    out per half, alternating sync/scalar rings. The last two mbs run
    h-sequentially so half0's copy+DMA overlap half1's chain (shorter
    tail).

Precision (gate 2e-2): bf16 main + fp8 on 8/32 k-tiles + fp8 LoRA delta
measured 1.921e-2 on the harness inputs (numpy sim matches HW to ~1e-6).
fp8 e4m3 DoubleRow measured ~234ns per 512-col pair vs 2x218ns bf16.
"""

import numpy as np
import ml_dtypes

# problem shape (hardcoded per harness contract)
T = 8192
HID = 4096
Q_SIZE = 4096
KV_SIZE = 1024
D = Q_SIZE + 2 * KV_SIZE  # 6144
S = 8
R = 16
NCORES = 8
P = 128

TC = T // NCORES          # 1024 tokens per core
MB = D // P               # 48 output row-blocks of 128
KA = HID // P             # 32 k-tiles
NP8 = 4                   # k-tile PAIRS of the main GEMM done in fp8 DoubleRow
KB = KA - 2 * NP8         # 24 k-tiles of the main GEMM done in bf16
NH = TC // 512            # 2 token halves (psum bank = 512 fp32)
JA = KA // 2              # 16 k-tile pairs
SP = np.float32(32768.0)  # psum scale SX*SA: bf16 W is pre-scaled by SP so
                          # fp8 (x*32)(W*1024) products accumulate coherently;
                          # the final psum->sbuf copy descales by 1/SP (exact)

_CACHE = {}


def _build_nc():
    import concourse.mybir as mybir
    import concourse.tile as tile
    from concourse import bacc

    bf16 = mybir.dt.bfloat16
    f32 = mybir.dt.float32
    f8 = mybir.dt.float8e4
    DR = mybir.MatmulPerfMode.DoubleRow

    nc = bacc.Bacc(None, target_bir_lowering=False, debug=False)

    # ---- DRAM parameters (per-core shapes)
    PX1 = 3 * P + 512  # pax1 row: 3 A targets then x8 half0
    x_d = nc.declare_dram_parameter("x_sh", [P, KB, TC], bf16, isOutput=False)
    w_d = nc.declare_dram_parameter("w_t", [MB, P, KB, P], bf16, isOutput=False)
    w8_d = nc.declare_dram_parameter("w8_t", [MB, P, NP8, 2, P], f8, isOutput=False)
    px1_d = nc.declare_dram_parameter("pax1", [P, JA, 2, PX1], f8, isOutput=False)
    px2_d = nc.declare_dram_parameter("pax2", [P, JA, 2, 512], f8, isOutput=False)
    b_d = nc.declare_dram_parameter("b_t", [P, MB, P], bf16, isOutput=False)
    g_d = nc.declare_dram_parameter("gate", [P, TC], f32, isOutput=False)
    y_d = nc.declare_dram_parameter("y_out", [MB, P, TC], f32, isOutput=True)

    with tile.TileContext(nc) as tc:
        with tc.tile_pool(name="xres", bufs=1) as xres_pool, \
             tc.tile_pool(name="wp", bufs=3) as w_pool, \
             tc.tile_pool(name="ab", bufs=1) as ab_pool, \
             tc.tile_pool(name="agp", bufs=1) as ag_pool, \
             tc.tile_pool(name="stp", bufs=3) as st_pool, \
             tc.tile_pool(name="psum", bufs=8, space="PSUM") as ps_pool:

            # resident operands
            x_res = xres_pool.tile([P, KB, TC], bf16, tag="xres")
            px1_t = ab_pool.tile([P, JA, 2, PX1], f8, tag="pax1")
            px2_t = ab_pool.tile([P, JA, 2, 512], f8, tag="pax2")
            b_t = ab_pool.tile([P, MB, P], bf16, tag="b")
            gate_t = ab_pool.tile([P, TC], f32, tag="gate")

            # ---- sync ring: x chunks, gate, b (split), even-mb outputs
            for (lo, hi) in [(0, 2), (2, 6)]:
                nc.sync.dma_start(out=x_res[:, lo:hi, :], in_=x_d[:, lo:hi, :])
            nc.sync.dma_start(out=gate_t[:], in_=g_d[:])
            for (lo, hi) in [(6, 10), (10, 14)]:
                nc.sync.dma_start(out=x_res[:, lo:hi, :], in_=x_d[:, lo:hi, :])
            nc.sync.dma_start(out=b_t[:, 0:8], in_=b_d[:, 0:8])
            for (lo, hi) in [(14, 18), (18, 21), (21, KB)]:
                nc.sync.dma_start(out=x_res[:, lo:hi, :], in_=x_d[:, lo:hi, :])
            nc.sync.dma_start(out=b_t[:, 8:MB], in_=b_d[:, 8:MB])

            def load_w(mb, queue):
                w_t = w_pool.tile([P, KB, P], bf16, tag="w", name=f"w{mb}")
                queue.dma_start(out=w_t[:], in_=w_d[mb])
                w8_t = w_pool.tile([P, NP8, 2, P], f8, tag="w8", name=f"w8{mb}")
                queue.dma_start(out=w8_t[:], in_=w8_d[mb])
                return w_t, w8_t

            # ---- scalar ring: w0 (split), pax1 j-chunks, w1, pax2, W stream
            w0_t = w_pool.tile([P, KB, P], bf16, tag="w", name="w0")
            nc.scalar.dma_start(out=w0_t[:, 0:6, :], in_=w_d[0, :, 0:6, :])
            nc.scalar.dma_start(out=px1_t[:, 0:1], in_=px1_d[:, 0:1])
            nc.scalar.dma_start(out=w0_t[:, 6:KB, :], in_=w_d[0, :, 6:KB, :])
            w80_t = w_pool.tile([P, NP8, 2, P], f8, tag="w8", name="w80")
            nc.scalar.dma_start(out=w80_t[:], in_=w8_d[0])
            nc.scalar.dma_start(out=px1_t[:, 1:2], in_=px1_d[:, 1:2])
            nc.scalar.dma_start(out=px1_t[:, 2:4], in_=px1_d[:, 2:4])
            w1_t, w81_t = load_w(1, nc.scalar)
            nc.scalar.dma_start(out=px1_t[:, 4:8], in_=px1_d[:, 4:8])
            nc.scalar.dma_start(out=px1_t[:, 8:12], in_=px1_d[:, 8:12])
            nc.scalar.dma_start(out=px1_t[:, 12:JA], in_=px1_d[:, 12:JA])
            nc.scalar.dma_start(out=px2_t[:, 0:8], in_=px2_d[:, 0:8])
            nc.scalar.dma_start(out=px2_t[:, 8:JA], in_=px2_d[:, 8:JA])

            # ---------------- PE emission ------------------------------------
            # Phase A current-half psums (3 banks) + mb0/mb1 (4) = 7 of 8.
            ps_a = [ps_pool.tile([P, 512], f32, tag="ps", name=f"ps_a{i}")
                    for i in range(3)]
            ps0 = [ps_pool.tile([P, 512], f32, tag="ps", name=f"pm0_{h}")
                   for h in range(NH)]
            ps1 = [ps_pool.tile([P, 512], f32, tag="ps", name=f"pm1_{h}")
                   for h in range(NH)]

            def a_jgroup(ps3, jlo, jhi, h):
                mv = (lambda j: px1_t[:, j, :, 3 * P:]) if h == 0 else \
                     (lambda j: px2_t[:, j, :, :])
                for j in range(jlo, jhi):
                    for i in range(3):
                        nc.tensor.matmul(
                            ps3[i][:],
                            px1_t[:, j, :, i * P:(i + 1) * P],
                            mv(j),
                            start=(j == 0), stop=(j == JA - 1),
                            perf_mode=DR,
                        )

            def mb_ksub(pss, w_t, klo, khi):
                for k in range(klo, khi):
                    for h in range(NH):
                        nc.tensor.matmul(
                            pss[h][:],
                            w_t[:, k, :],
                            x_res[:, k, h * 512:(h + 1) * 512],
                            start=(k == 0), stop=False,
                        )

            def mb_fp8(pss, w8_t):
                for jp in range(NP8):
                    j = KB // 2 + jp
                    for h in range(NH):
                        mv = px1_t[:, j, :, 3 * P:] if h == 0 else px2_t[:, j, :, :]
                        nc.tensor.matmul(
                            pss[h][:],
                            w8_t[:, jp, :, :],
                            mv,
                            start=False, stop=False,
                            perf_mode=DR,
                        )

            ag = [ag_pool.tile([P, TC], bf16, tag=f"ag{i}", name=f"ag{i}")
                  for i in range(3)]

            def gate_half(ps3, h):
                sl = slice(h * 512, (h + 1) * 512)
                for i in range(3):
                    nc.vector.tensor_mul(ag[i][:, sl], ps3[i][:], gate_t[:, sl])

            # Warmup interleave, ordered to match DMA arrivals on both rings.
            mb_ksub(ps0, w0_t, 0, 2)
            a_jgroup(ps_a, 0, 1, 0)
            mb_ksub(ps0, w0_t, 2, 4)
            a_jgroup(ps_a, 1, 2, 0)
            mb_ksub(ps0, w0_t, 4, 6)
            a_jgroup(ps_a, 2, 4, 0)
            mb_ksub(ps0, w0_t, 6, 10)
            a_jgroup(ps_a, 4, 8, 0)
            mb_ksub(ps0, w0_t, 10, 14)
            a_jgroup(ps_a, 8, 12, 0)
            mb_ksub(ps0, w0_t, 14, 18)
            mb_ksub(ps1, w1_t, 0, 4)
            a_jgroup(ps_a, 12, JA, 0)
            mb_ksub(ps0, w0_t, 18, KB)
            gate_half(ps_a, 0)
            mb_ksub(ps1, w1_t, 4, 10)
            # Phase A half1 (reuses the 3 A banks after the gate reads them)
            ps_a1 = [ps_pool.tile([P, 512], f32, tag="ps", name=f"ps_b{i}")
                     for i in range(3)]
            a_jgroup(ps_a1, 0, 8, 1)
            mb_ksub(ps1, w1_t, 10, 16)
            a_jgroup(ps_a1, 8, JA, 1)
            mb_ksub(ps1, w1_t, 16, KB)
            gate_half(ps_a1, 1)
            mb_fp8(ps0, w80_t)
            mb_fp8(ps1, w81_t)

            def finish_mb(mb, pss, h_list=None):
                i = 0 if mb < Q_SIZE // P else (1 if mb < (Q_SIZE + KV_SIZE) // P else 2)
                st = st_pool.tile([P, TC], f32, tag="st", name=f"st{mb}")
                oq = nc.sync if mb % 2 == 0 else nc.scalar
                for h in (h_list if h_list is not None else range(NH)):
                    nc.tensor.matmul(
                        pss[h][:],
                        b_t[:, mb, :],
                        ag[i][:, h * 512:(h + 1) * 512],
                        start=False, stop=True,
                    )
                    nc.vector.tensor_scalar_mul(
                        st[:, h * 512:(h + 1) * 512], pss[h][:],
                        float(1.0 / SP))
                    oq.dma_start(
                        out=y_d[mb, :, h * 512:(h + 1) * 512],
                        in_=st[:, h * 512:(h + 1) * 512],
                    )
                return st

            finish_mb(0, ps0)
            finish_mb(1, ps1)

            # ------------- Phase C: remaining mb chains ---------------------
            for mb in range(2, MB):
                w_t, w8_t = load_w(mb, nc.scalar)
                pss = [
                    ps_pool.tile([P, 512], f32, tag="ps", name=f"pm{mb}_{h}")
                    for h in range(NH)
                ]
                if mb < MB - 2:
                    mb_ksub(pss, w_t, 0, KB)
                    mb_fp8(pss, w8_t)
                    finish_mb(mb, pss)
                else:
                    # h-sequential tail: half0's copy+DMA overlap half1's chain
                    i = 0 if mb < Q_SIZE // P else (1 if mb < (Q_SIZE + KV_SIZE) // P else 2)
                    st = st_pool.tile([P, TC], f32, tag="st", name=f"st{mb}")
                    oq = nc.sync if mb % 2 == 0 else nc.scalar
                    for h in range(NH):
                        mvx = lambda j: (px1_t[:, j, :, 3 * P:] if h == 0
                                         else px2_t[:, j, :, :])
                        for k in range(KB):
                            nc.tensor.matmul(
                                pss[h][:], w_t[:, k, :],
                                x_res[:, k, h * 512:(h + 1) * 512],
                                start=(k == 0), stop=False,
                            )
                        for jp in range(NP8):
                            nc.tensor.matmul(
                                pss[h][:], w8_t[:, jp, :, :], mvx(KB // 2 + jp),
                                start=False, stop=False, perf_mode=DR,
                            )
                        nc.tensor.matmul(
                            pss[h][:], b_t[:, mb, :],
                            ag[i][:, h * 512:(h + 1) * 512],
                            start=False, stop=True,
                        )
                        nc.vector.tensor_scalar_mul(
                            st[:, h * 512:(h + 1) * 512], pss[h][:],
                            float(1.0 / SP))
                        for (lo, hi) in [(0, 256), (256, 512)]:
                            oq.dma_start(
                                out=y_d[mb, :, h * 512 + lo:h * 512 + hi],
                                in_=st[:, h * 512 + lo:h * 512 + hi],
                            )

    nc.compile()
    return nc


def _get_nc():
    if "nc" not in _CACHE:
        _CACHE["nc"] = _build_nc()
    return _CACHE["nc"]


def _prep_in_maps(x, W, lora_A, lora_B_q, lora_B_k, lora_B_v, scaling, token_to_slot):
    f = np.float32
    bf = ml_dtypes.bfloat16
    x = np.ascontiguousarray(x, dtype=f)
    W = np.ascontiguousarray(W, dtype=f)

    # x shard, moving operand: [c, p, ka, tl]  (h = ka*128 + p, t = c*1024 + tl)
    x_f32 = np.ascontiguousarray(
        x.reshape(NCORES, TC, KA, P).transpose(0, 3, 2, 1))
    x_sh = np.ascontiguousarray(x_f32[:, :, :KB, :]).astype(bf)
    # W stationary: [mb, p, ka, dl]  (d = mb*128 + dl)  -- replicated.
    # bf16 part pre-scaled by SP to match the fp8 psum scale; the last
    # 2*NP8 k-tiles go as fp8(W*1024) DoubleRow pairs.
    w_all = W.reshape(MB, P, KA, P).transpose(0, 3, 2, 1)
    w_t = np.ascontiguousarray(w_all[:, :, :KB, :] * SP).astype(bf)
    # fp8 e4m3 copies for the LoRA down-proj (DoubleRow pairs of k-tiles);
    # pax1 = [A targets | x8 half0], pax2 = [x8 half1]. 1/(SX*SA) descale
    # folds into the gate below.
    SX, SA = np.float32(32.0), np.float32(1024.0)
    f8 = ml_dtypes.float8_e4m3
    w8 = np.ascontiguousarray(
        (w_all[:, :, KB:, :] * SA).astype(f8).reshape(MB, P, NP8, 2, P))
    x8 = (x_f32 * SX).astype(f8).reshape(NCORES, P, JA, 2, TC)
    a_f32 = np.ascontiguousarray(
        np.asarray(lora_A, dtype=f).reshape(S, 3, R, KA, P).transpose(4, 3, 1, 0, 2)
        .reshape(P, KA, 3, S * R))
    a8 = (a_f32 * SA).astype(f8).reshape(P, JA, 2, 3 * S * R)
    pax1 = np.concatenate(
        [np.broadcast_to(a8, (NCORES,) + a8.shape), x8[..., 0:512]], axis=-1)
    pax1 = np.ascontiguousarray(pax1)
    pax2 = np.ascontiguousarray(x8[..., 512:])
    # LoRA B stationary: [(s r), mb, dl] -- replicated
    bq = np.asarray(lora_B_q, dtype=f).transpose(0, 2, 1).reshape(S * R, Q_SIZE)
    bk = np.asarray(lora_B_k, dtype=f).transpose(0, 2, 1).reshape(S * R, KV_SIZE)
    bv = np.asarray(lora_B_v, dtype=f).transpose(0, 2, 1).reshape(S * R, KV_SIZE)
    b_t = np.ascontiguousarray(
        np.concatenate([bq, bk, bv], axis=1).reshape(S * R, MB, P)).astype(bf)
    # routing gate, expanded over ranks: [c, (s r), tl]. The LoRA psum is
    # already SP x true scale (x*32 times A*1024), which matches the main
    # psum scale, so the gate is just the per-slot scaling.
    slot = np.asarray(token_to_slot).reshape(NCORES, TC)
    g = (slot[:, None, :] == np.arange(S, dtype=slot.dtype)[None, :, None])
    g = g.astype(f) * np.asarray(scaling, dtype=f)[None, :, None]
    gate = np.ascontiguousarray(np.repeat(g, R, axis=1))

    in_maps = []
    for c in range(NCORES):
        in_maps.append({
            "x_sh": x_sh[c],
            "w_t": w_t,
            "w8_t": w8,
            "pax1": pax1[c],
            "pax2": pax2[c],
            "b_t": b_t,
            "gate": gate[c],
        })
    return in_maps


def _assemble(results):
    out = np.empty((T, D), dtype=np.float32)
    for c in range(NCORES):
        out[c * TC:(c + 1) * TC, :] = results[c]["y_out"].reshape(D, TC).T
    return out


def _run(inputs, trace=False):
    from concourse.bass_utils import run_bass_kernel_spmd
    nc = _get_nc()
    in_maps = _prep_in_maps(**inputs)
    res = run_bass_kernel_spmd(
        nc, in_maps, core_ids=list(range(NCORES)), trace=trace)
    return res


def kernel(**inputs) -> np.ndarray:
    res = _run(inputs, trace=False)
    return _assemble(res.results)


if __name__ == "__main__":
    rng = np.random.default_rng(0)
    ins = {
        "x": rng.standard_normal((T, HID)).astype(np.float32),
        "W": (rng.standard_normal((D, HID)) * 0.02).astype(np.float32),
        "lora_A": (rng.standard_normal((S, 3, R, HID)) * 0.02).astype(np.float32),
        "lora_B_q": (rng.standard_normal((S, Q_SIZE, R)) * 0.02).astype(np.float32),
        "lora_B_k": (rng.standard_normal((S, KV_SIZE, R)) * 0.02).astype(np.float32),
        "lora_B_v": (rng.standard_normal((S, KV_SIZE, R)) * 0.02).astype(np.float32),
        "scaling": rng.uniform(0.5, 2.0, S).astype(np.float32),
        "token_to_slot": rng.integers(0, S, T).astype(np.int32),
    }
    out = kernel(**ins)
    print("out", out.shape, out.dtype)


# revision 12
# speedup vs baseline: 1.0212x; 1.0014x over previous
"""LoRA QKV fused projection kernel for 8 TRN2 NeuronCores.

Reference computation (T=8192 tokens, HID=4096, D=6144 out, S=8 slots, R=16):
    y = x @ W.T
    a[t,s,i,r] = sum_h x[t,h] * lora_A[s,i,r,h]         (down-proj, all slots)
    a *= onehot(token_to_slot)[t,s] * scaling[s]         (routing gate)
    d[t, :] = concat_i( sum_{s,r} a[t,s,i,r] * B_i[s,:,r] )   (up-proj)
    out = y + d
Sharding: data-parallel over tokens; core c owns tokens [c*1024, (c+1)*1024).

Per-core dataflow (v5):
  * Phase A (LoRA down-proj aT = A @ x) in fp8 e4m3 DoubleRow over all 16
    k-tile PAIRS, one token half at a time (3 psum banks), j-major, paced
    by the pax tables streaming on the scalar ring. pax is split into
    pax1 = [A targets | x8 half0] and pax2 = [x8 half1] so half0 can
    start early.
  * mb0 and mb1's bf16 k-chains interleave BETWEEN Phase A j-groups
    (banks: 3 A + 2+2 = 7 of 8); their fp8 pairs + B-matmuls run right
    after the gate. PE stays fed from ~5us while ~14MB of warmup DMA
    (w0/w1, x, pax, gate) streams on two rings (gpsimd as a third ring
    measured WORSE: its DRAIN ops add ~14us of idle).
  * Phase B: routing gate (host-built onehot*scaling, expanded over rank)
    applied on DVE per half: ag = psum_a * gate, written bf16.
  * Phase C per output row-block mb (48): 24 bf16 k-tiles (W pre-scaled by
    SP=2^15 to match the fp8 psum scale) + 4 fp8 DoubleRow k-tile PAIRS
    accumulate into 2 psum tiles (token halves); the LoRA up-proj
    B[mb] @ ag[i] accumulates INTO THE SAME psum (start=False), fusing
    base + delta. psum->sbuf copy descales by 1/SP exactly, then one DMA
    out per half, alternating sync/scalar rings. The last two mbs run
    h-sequentially so half0's copy+DMA overlap half1's chain (shorter
    tail).

Precision (gate 2e-2): bf16 main + fp8 on 8/32 k-tiles + fp8 LoRA delta
measured 1.921e-2 on the harness inputs (numpy sim matches HW to ~1e-6).
fp8 e4m3 DoubleRow measured ~234ns per 512-col pair vs 2x218ns bf16.
"""

import numpy as np
import ml_dtypes

# problem shape (hardcoded per harness contract)
T = 8192
HID = 4096
Q_SIZE = 4096
KV_SIZE = 1024
D = Q_SIZE + 2 * KV_SIZE  # 6144
S = 8
R = 16
NCORES = 8
P = 128

TC = T // NCORES          # 1024 tokens per core
MB = D // P               # 48 output row-blocks of 128
KA = HID // P             # 32 k-tiles
NP8 = 4                   # k-tile PAIRS of the main GEMM done in fp8 DoubleRow
KB = KA - 2 * NP8         # 24 k-tiles of the main GEMM done in bf16
NH = TC // 512            # 2 token halves (psum bank = 512 fp32)
JA = KA // 2              # 16 k-tile pairs
SP = np.float32(32768.0)  # psum scale SX*SA: bf16 W is pre-scaled by SP so
                          # fp8 (x*32)(W*1024) products accumulate coherently;
                          # the final psum->sbuf copy descales by 1/SP (exact)

_CACHE = {}


def _build_nc():
    import concourse.mybir as mybir
    import concourse.tile as tile
    from concourse import bacc

    bf16 = mybir.dt.bfloat16
    f32 = mybir.dt.float32
    f8 = mybir.dt.float8e4
    DR = mybir.MatmulPerfMode.DoubleRow

    nc = bacc.Bacc(None, target_bir_lowering=False, debug=False)

    # ---- DRAM parameters (per-core shapes)
    PX1 = 3 * P + 512  # pax1 row: 3 A targets then x8 half0
    x_d = nc.declare_dram_parameter("x_sh", [P, KB, TC], bf16, isOutput=False)
    w_d = nc.declare_dram_parameter("w_t", [MB, P, KB, P], bf16, isOutput=False)
    w8_d = nc.declare_dram_parameter("w8_t", [MB, P, NP8, 2, P], f8, isOutput=False)
    px1_d = nc.declare_dram_parameter("pax1", [P, JA, 2, PX1], f8, isOutput=False)
    px2_d = nc.declare_dram_parameter("pax2", [P, JA, 2, 512], f8, isOutput=False)
    b_d = nc.declare_dram_parameter("b_t", [P, MB, P], bf16, isOutput=False)
    g_d = nc.declare_dram_parameter("gate", [P, TC], f32, isOutput=False)
    y_d = nc.declare_dram_parameter("y_out", [MB, P, TC], f32, isOutput=True)

    with tile.TileContext(nc) as tc:
        with tc.tile_pool(name="xres", bufs=1) as xres_pool, \
             tc.tile_pool(name="wp", bufs=3) as w_pool, \
             tc.tile_pool(name="ab", bufs=1) as ab_pool, \
             tc.tile_pool(name="agp", bufs=1) as ag_pool, \
             tc.tile_pool(name="stp", bufs=3) as st_pool, \
             tc.tile_pool(name="psum", bufs=8, space="PSUM") as ps_pool:

            # resident operands
            x_res = xres_pool.tile([P, KB, TC], bf16, tag="xres")
            px1_t = ab_pool.tile([P, JA, 2, PX1], f8, tag="pax1")
            px2_t = ab_pool.tile([P, JA, 2, 512], f8, tag="pax2")
            b_t = ab_pool.tile([P, MB, P], bf16, tag="b")
            gate_t = ab_pool.tile([P, TC], f32, tag="gate")

            # ---- sync ring: x chunks, gate, b (split), even-mb outputs
            for (lo, hi) in [(0, 2), (2, 6), (6, 10), (10, 14), (14, 18)]:
                nc.sync.dma_start(out=x_res[:, lo:hi, :], in_=x_d[:, lo:hi, :])
            nc.sync.dma_start(out=gate_t[:], in_=g_d[:])
            nc.sync.dma_start(out=x_res[:, 18:21, :], in_=x_d[:, 18:21, :])
            nc.sync.dma_start(out=b_t[:, 0:8], in_=b_d[:, 0:8])
            nc.sync.dma_start(out=x_res[:, 21:KB, :], in_=x_d[:, 21:KB, :])
            nc.sync.dma_start(out=b_t[:, 8:MB], in_=b_d[:, 8:MB])

            def load_w(mb, queue):
                w_t = w_pool.tile([P, KB, P], bf16, tag="w", name=f"w{mb}")
                queue.dma_start(out=w_t[:], in_=w_d[mb])
                w8_t = w_pool.tile([P, NP8, 2, P], f8, tag="w8", name=f"w8{mb}")
                queue.dma_start(out=w8_t[:], in_=w8_d[mb])
                return w_t, w8_t

            # ---- scalar ring: w0 (split), pax1 j-chunks, w1 (split), pax2,
            # then the W stream. Ordered against the PE interleave below.
            w0_t = w_pool.tile([P, KB, P], bf16, tag="w", name="w0")
            nc.scalar.dma_start(out=w0_t[:, 0:6, :], in_=w_d[0, :, 0:6, :])
            nc.scalar.dma_start(out=px1_t[:, 0:1], in_=px1_d[:, 0:1])
            nc.scalar.dma_start(out=px1_t[:, 1:2], in_=px1_d[:, 1:2])
            nc.scalar.dma_start(out=w0_t[:, 6:KB, :], in_=w_d[0, :, 6:KB, :])
            w80_t = w_pool.tile([P, NP8, 2, P], f8, tag="w8", name="w80")
            nc.scalar.dma_start(out=w80_t[:], in_=w8_d[0])
            nc.scalar.dma_start(out=px1_t[:, 2:4], in_=px1_d[:, 2:4])
            nc.scalar.dma_start(out=px1_t[:, 4:6], in_=px1_d[:, 4:6])
            nc.scalar.dma_start(out=px1_t[:, 6:8], in_=px1_d[:, 6:8])
            nc.scalar.dma_start(out=px1_t[:, 8:10], in_=px1_d[:, 8:10])
            nc.scalar.dma_start(out=px1_t[:, 10:12], in_=px1_d[:, 10:12])
            w1_t = w_pool.tile([P, KB, P], bf16, tag="w", name="w1")
            nc.scalar.dma_start(out=w1_t[:, 0:6, :], in_=w_d[1, :, 0:6, :])
            nc.scalar.dma_start(out=px1_t[:, 12:14], in_=px1_d[:, 12:14])
            nc.scalar.dma_start(out=px1_t[:, 14:JA], in_=px1_d[:, 14:JA])
            nc.scalar.dma_start(out=w1_t[:, 6:KB, :], in_=w_d[1, :, 6:KB, :])
            w81_t = w_pool.tile([P, NP8, 2, P], f8, tag="w8", name="w81")
            nc.scalar.dma_start(out=w81_t[:], in_=w8_d[1])
            nc.scalar.dma_start(out=px2_t[:, 0:8], in_=px2_d[:, 0:8])
            nc.scalar.dma_start(out=px2_t[:, 8:JA], in_=px2_d[:, 8:JA])

            # ---------------- PE emission ------------------------------------
            # Phase A current-half psums (3 banks) + mb0/mb1 (4) = 7 of 8.
            ps_a = [ps_pool.tile([P, 512], f32, tag="ps", name=f"ps_a{i}")
                    for i in range(3)]
            ps0 = [ps_pool.tile([P, 512], f32, tag="ps", name=f"pm0_{h}")
                   for h in range(NH)]
            ps1 = [ps_pool.tile([P, 512], f32, tag="ps", name=f"pm1_{h}")
                   for h in range(NH)]

            def a_jgroup(ps3, jlo, jhi, h):
                mv = (lambda j: px1_t[:, j, :, 3 * P:]) if h == 0 else \
                     (lambda j: px2_t[:, j, :, :])
                for j in range(jlo, jhi):
                    for i in range(3):
                        nc.tensor.matmul(
                            ps3[i][:],
                            px1_t[:, j, :, i * P:(i + 1) * P],
                            mv(j),
                            start=(j == 0), stop=(j == JA - 1),
                            perf_mode=DR,
                        )

            def mb_ksub(pss, w_t, klo, khi):
                for k in range(klo, khi):
                    for h in range(NH):
                        nc.tensor.matmul(
                            pss[h][:],
                            w_t[:, k, :],
                            x_res[:, k, h * 512:(h + 1) * 512],
                            start=(k == 0), stop=False,
                        )

            def mb_fp8(pss, w8_t):
                for jp in range(NP8):
                    j = KB // 2 + jp
                    for h in range(NH):
                        mv = px1_t[:, j, :, 3 * P:] if h == 0 else px2_t[:, j, :, :]
                        nc.tensor.matmul(
                            pss[h][:],
                            w8_t[:, jp, :, :],
                            mv,
                            start=False, stop=False,
                            perf_mode=DR,
                        )

            ag = [ag_pool.tile([P, TC], bf16, tag=f"ag{i}", name=f"ag{i}")
                  for i in range(3)]

            def gate_half(ps3, h):
                sl = slice(h * 512, (h + 1) * 512)
                for i in range(3):
                    nc.vector.tensor_mul(ag[i][:, sl], ps3[i][:], gate_t[:, sl])

            # Warmup interleave, fine bites ordered to match DMA arrivals on
            # both rings (A j-pairs vs pax1 chunks, mb0/mb1 k-bites vs x/w).
            mb_ksub(ps0, w0_t, 0, 2)
            a_jgroup(ps_a, 0, 2, 0)
            mb_ksub(ps0, w0_t, 2, 4)
            a_jgroup(ps_a, 2, 4, 0)
            mb_ksub(ps0, w0_t, 4, 6)
            a_jgroup(ps_a, 4, 6, 0)
            mb_ksub(ps0, w0_t, 6, 8)
            a_jgroup(ps_a, 6, 8, 0)
            mb_ksub(ps0, w0_t, 8, 10)
            a_jgroup(ps_a, 8, 10, 0)
            mb_ksub(ps0, w0_t, 10, 12)
            a_jgroup(ps_a, 10, 12, 0)
            mb_ksub(ps0, w0_t, 12, 14)
            mb_ksub(ps1, w1_t, 0, 4)
            a_jgroup(ps_a, 12, 14, 0)
            mb_ksub(ps0, w0_t, 14, 18)
            a_jgroup(ps_a, 14, JA, 0)
            gate_half(ps_a, 0)
            mb_ksub(ps0, w0_t, 18, KB)
            mb_ksub(ps1, w1_t, 4, 10)
            # Phase A half1 (reuses the 3 A banks after the gate reads them)
            ps_a1 = [ps_pool.tile([P, 512], f32, tag="ps", name=f"ps_b{i}")
                     for i in range(3)]
            a_jgroup(ps_a1, 0, 8, 1)
            mb_ksub(ps1, w1_t, 10, 16)
            a_jgroup(ps_a1, 8, JA, 1)
            mb_ksub(ps1, w1_t, 16, KB)
            gate_half(ps_a1, 1)
            mb_fp8(ps0, w80_t)
            mb_fp8(ps1, w81_t)

            def finish_mb(mb, pss, h_list=None):
                i = 0 if mb < Q_SIZE // P else (1 if mb < (Q_SIZE + KV_SIZE) // P else 2)
                st = st_pool.tile([P, TC], f32, tag="st", name=f"st{mb}")
                oq = nc.sync if mb % 2 == 0 else nc.scalar
                for h in (h_list if h_list is not None else range(NH)):
                    nc.tensor.matmul(
                        pss[h][:],
                        b_t[:, mb, :],
                        ag[i][:, h * 512:(h + 1) * 512],
                        start=False, stop=True,
                    )
                    nc.vector.tensor_scalar_mul(
                        st[:, h * 512:(h + 1) * 512], pss[h][:],
                        float(1.0 / SP))
                    oq.dma_start(
                        out=y_d[mb, :, h * 512:(h + 1) * 512],
                        in_=st[:, h * 512:(h + 1) * 512],
                    )
                return st

            finish_mb(0, ps0)
            finish_mb(1, ps1)

            # ------------- Phase C: remaining mb chains ---------------------
            # h-sequential: one psum bank per half, half0's B/copy/DMA overlap
            # half1's chain; last mb's final DMAs split to shorten the tail.
            for mb in range(2, MB):
                w_t, w8_t = load_w(mb, nc.scalar)
                i = 0 if mb < Q_SIZE // P else (1 if mb < (Q_SIZE + KV_SIZE) // P else 2)
                st = st_pool.tile([P, TC], f32, tag="st", name=f"st{mb}")
                oq = nc.sync if mb % 2 == 0 else nc.scalar
                pieces = 2 if mb >= MB - 2 else 1
                for h in range(NH):
                    ps = ps_pool.tile([P, 512], f32, tag="ps", name=f"pm{mb}_{h}")
                    for k in range(KB):
                        nc.tensor.matmul(
                            ps[:], w_t[:, k, :],
                            x_res[:, k, h * 512:(h + 1) * 512],
                            start=(k == 0), stop=False,
                        )
                    for jp in range(NP8):
                        j = KB // 2 + jp
                        mv = px1_t[:, j, :, 3 * P:] if h == 0 else px2_t[:, j, :, :]
                        nc.tensor.matmul(
                            ps[:], w8_t[:, jp, :, :], mv,
                            start=False, stop=False, perf_mode=DR,
                        )
                    nc.tensor.matmul(
                        ps[:], b_t[:, mb, :],
                        ag[i][:, h * 512:(h + 1) * 512],
                        start=False, stop=True,
                    )
                    nc.vector.tensor_scalar_mul(
                        st[:, h * 512:(h + 1) * 512], ps[:],
                        float(1.0 / SP))
                    pw = 512 // pieces
                    for pc in range(pieces):
                        lo = h * 512 + pc * pw
                        oq.dma_start(
                            out=y_d[mb, :, lo:lo + pw],
                            in_=st[:, lo:lo + pw],
                        )

    nc.compile()
    return nc


def _get_nc():
    if "nc" not in _CACHE:
        _CACHE["nc"] = _build_nc()
    return _CACHE["nc"]


def _prep_in_maps(x, W, lora_A, lora_B_q, lora_B_k, lora_B_v, scaling, token_to_slot):
    f = np.float32
    bf = ml_dtypes.bfloat16
    x = np.ascontiguousarray(x, dtype=f)
    W = np.ascontiguousarray(W, dtype=f)

    # x shard, moving operand: [c, p, ka, tl]  (h = ka*128 + p, t = c*1024 + tl)
    x_f32 = np.ascontiguousarray(
        x.reshape(NCORES, TC, KA, P).transpose(0, 3, 2, 1))
    x_sh = np.ascontiguousarray(x_f32[:, :, :KB, :]).astype(bf)
    # W stationary: [mb, p, ka, dl]  (d = mb*128 + dl)  -- replicated.
    # bf16 part pre-scaled by SP to match the fp8 psum scale; the last
    # 2*NP8 k-tiles go as fp8(W*1024) DoubleRow pairs.
    w_all = W.reshape(MB, P, KA, P).transpose(0, 3, 2, 1)
    w_t = np.ascontiguousarray(w_all[:, :, :KB, :] * SP).astype(bf)
    # fp8 e4m3 copies for the LoRA down-proj (DoubleRow pairs of k-tiles);
    # pax1 = [A targets | x8 half0], pax2 = [x8 half1]. 1/(SX*SA) descale
    # folds into the gate below.
    SX, SA = np.float32(32.0), np.float32(1024.0)
    f8 = ml_dtypes.float8_e4m3
    w8 = np.ascontiguousarray(
        (w_all[:, :, KB:, :] * SA).astype(f8).reshape(MB, P, NP8, 2, P))
    x8 = (x_f32 * SX).astype(f8).reshape(NCORES, P, JA, 2, TC)
    a_f32 = np.ascontiguousarray(
        np.asarray(lora_A, dtype=f).reshape(S, 3, R, KA, P).transpose(4, 3, 1, 0, 2)
        .reshape(P, KA, 3, S * R))
    a8 = (a_f32 * SA).astype(f8).reshape(P, JA, 2, 3 * S * R)
    pax1 = np.concatenate(
        [np.broadcast_to(a8, (NCORES,) + a8.shape), x8[..., 0:512]], axis=-1)
    pax1 = np.ascontiguousarray(pax1)
    pax2 = np.ascontiguousarray(x8[..., 512:])
    # LoRA B stationary: [(s r), mb, dl] -- replicated
    bq = np.asarray(lora_B_q, dtype=f).transpose(0, 2, 1).reshape(S * R, Q_SIZE)
    bk = np.asarray(lora_B_k, dtype=f).transpose(0, 2, 1).reshape(S * R, KV_SIZE)
    bv = np.asarray(lora_B_v, dtype=f).transpose(0, 2, 1).reshape(S * R, KV_SIZE)
    b_t = np.ascontiguousarray(
        np.concatenate([bq, bk, bv], axis=1).reshape(S * R, MB, P)).astype(bf)
    # routing gate, expanded over ranks: [c, (s r), tl]. The LoRA psum is
    # already SP x true scale (x*32 times A*1024), which matches the main
    # psum scale, so the gate is just the per-slot scaling.
    slot = np.asarray(token_to_slot).reshape(NCORES, TC)
    g = (slot[:, None, :] == np.arange(S, dtype=slot.dtype)[None, :, None])
    g = g.astype(f) * np.asarray(scaling, dtype=f)[None, :, None]
    gate = np.ascontiguousarray(np.repeat(g, R, axis=1))

    in_maps = []
    for c in range(NCORES):
        in_maps.append({
            "x_sh": x_sh[c],
            "w_t": w_t,
            "w8_t": w8,
            "pax1": pax1[c],
            "pax2": pax2[c],
            "b_t": b_t,
            "gate": gate[c],
        })
    return in_maps


def _assemble(results):
    out = np.empty((T, D), dtype=np.float32)
    for c in range(NCORES):
        out[c * TC:(c + 1) * TC, :] = results[c]["y_out"].reshape(D, TC).T
    return out


def _run(inputs, trace=False):
    from concourse.bass_utils import run_bass_kernel_spmd
    nc = _get_nc()
    in_maps = _prep_in_maps(**inputs)
    res = run_bass_kernel_spmd(
        nc, in_maps, core_ids=list(range(NCORES)), trace=trace)
    return res


def kernel(**inputs) -> np.ndarray:
    res = _run(inputs, trace=False)
    return _assemble(res.results)


if __name__ == "__main__":
    rng = np.random.default_rng(0)
    ins = {
        "x": rng.standard_normal((T, HID)).astype(np.float32),
        "W": (rng.standard_normal((D, HID)) * 0.02).astype(np.float32),
        "lora_A": (rng.standard_normal((S, 3, R, HID)) * 0.02).astype(np.float32),
        "lora_B_q": (rng.standard_normal((S, Q_SIZE, R)) * 0.02).astype(np.float32),
        "lora_B_k": (rng.standard_normal((S, KV_SIZE, R)) * 0.02).astype(np.float32),
        "lora_B_v": (rng.standard_normal((S, KV_SIZE, R)) * 0.02).astype(np.float32),
        "scaling": rng.uniform(0.5, 2.0, S).astype(np.float32),
        "token_to_slot": rng.integers(0, S, T).astype(np.int32),
    }
    out = kernel(**ins)
    print("out", out.shape, out.dtype)


# revision 14
# speedup vs baseline: 1.0272x; 1.0058x over previous
"""LoRA QKV fused projection kernel for 8 TRN2 NeuronCores.

Reference computation (T=8192 tokens, HID=4096, D=6144 out, S=8 slots, R=16):
    y = x @ W.T
    a[t,s,i,r] = sum_h x[t,h] * lora_A[s,i,r,h]         (down-proj, all slots)
    a *= onehot(token_to_slot)[t,s] * scaling[s]         (routing gate)
    d[t, :] = concat_i( sum_{s,r} a[t,s,i,r] * B_i[s,:,r] )   (up-proj)
    out = y + d
Sharding: data-parallel over tokens; core c owns tokens [c*1024, (c+1)*1024).

Per-core dataflow (v5):
  * Phase A (LoRA down-proj aT = A @ x) in fp8 e4m3 DoubleRow over all 16
    k-tile PAIRS, one token half at a time (3 psum banks), j-major, paced
    by the pax tables streaming on the scalar ring. pax is split into
    pax1 = [A targets | x8 half0] and pax2 = [x8 half1] so half0 can
    start early.
  * mb0 and mb1's bf16 k-chains interleave BETWEEN Phase A j-groups
    (banks: 3 A + 2+2 = 7 of 8); their fp8 pairs + B-matmuls run right
    after the gate. PE stays fed from ~5us while ~14MB of warmup DMA
    (w0/w1, x, pax, gate) streams on two rings (gpsimd as a third ring
    measured WORSE: its DRAIN ops add ~14us of idle).
  * Phase B: routing gate (host-built onehot*scaling, expanded over rank)
    applied on DVE per half: ag = psum_a * gate, written bf16.
  * Phase C per output row-block mb (48): 24 bf16 k-tiles (W pre-scaled by
    SP=2^15 to match the fp8 psum scale) + 4 fp8 DoubleRow k-tile PAIRS
    accumulate into 2 psum tiles (token halves); the LoRA up-proj
    B[mb] @ ag[i] accumulates INTO THE SAME psum (start=False), fusing
    base + delta. psum->sbuf copy descales by 1/SP exactly, then one DMA
    out per half, alternating sync/scalar rings. The last two mbs run
    h-sequentially so half0's copy+DMA overlap half1's chain (shorter
    tail).

Precision (gate 2e-2): bf16 main + fp8 on 8/32 k-tiles + fp8 LoRA delta
measured 1.921e-2 on the harness inputs (numpy sim matches HW to ~1e-6).
fp8 e4m3 DoubleRow measured ~234ns per 512-col pair vs 2x218ns bf16.
"""

import numpy as np
import ml_dtypes

# problem shape (hardcoded per harness contract)
T = 8192
HID = 4096
Q_SIZE = 4096
KV_SIZE = 1024
D = Q_SIZE + 2 * KV_SIZE  # 6144
S = 8
R = 16
NCORES = 8
P = 128

TC = T // NCORES          # 1024 tokens per core
MB = D // P               # 48 output row-blocks of 128
KA = HID // P             # 32 k-tiles
NP8 = 4                   # k-tile PAIRS of the main GEMM done in fp8 DoubleRow
KB = KA - 2 * NP8         # 24 k-tiles of the main GEMM done in bf16
NH = TC // 512            # 2 token halves (psum bank = 512 fp32)
JA = KA // 2              # 16 k-tile pairs
SP = np.float32(32768.0)  # psum scale SX*SA: bf16 W is pre-scaled by SP so
                          # fp8 (x*32)(W*1024) products accumulate coherently;
                          # the final psum->sbuf copy descales by 1/SP (exact)

_CACHE = {}


def _build_nc():
    import concourse.mybir as mybir
    import concourse.tile as tile
    from concourse import bacc

    bf16 = mybir.dt.bfloat16
    f32 = mybir.dt.float32
    f8 = mybir.dt.float8e4
    DR = mybir.MatmulPerfMode.DoubleRow

    nc = bacc.Bacc(None, target_bir_lowering=False, debug=False)

    # ---- DRAM parameters (per-core shapes)
    PX1 = 3 * P + 512  # pax1 row: 3 A targets then x8 half0
    x_d = nc.declare_dram_parameter("x_sh", [P, KB, TC], bf16, isOutput=False)
    w_d = nc.declare_dram_parameter("w_t", [MB, P, KB, P], bf16, isOutput=False)
    w8_d = nc.declare_dram_parameter("w8_t", [MB, P, NP8, 2, P], f8, isOutput=False)
    px1_d = nc.declare_dram_parameter("pax1", [P, JA, 2, PX1], f8, isOutput=False)
    px2_d = nc.declare_dram_parameter("pax2", [P, JA, 2, 512], f8, isOutput=False)
    b_d = nc.declare_dram_parameter("b_t", [P, MB, P], bf16, isOutput=False)
    g_d = nc.declare_dram_parameter("gate", [P, TC], f32, isOutput=False)
    y_d = nc.declare_dram_parameter("y_out", [MB, P, TC], f32, isOutput=True)

    with tile.TileContext(nc) as tc:
        with tc.tile_pool(name="xres", bufs=1) as xres_pool, \
             tc.tile_pool(name="wp", bufs=3) as w_pool, \
             tc.tile_pool(name="ab", bufs=1) as ab_pool, \
             tc.tile_pool(name="agp", bufs=1) as ag_pool, \
             tc.tile_pool(name="stp", bufs=3) as st_pool, \
             tc.tile_pool(name="psum", bufs=8, space="PSUM") as ps_pool:

            # resident operands
            x_res = xres_pool.tile([P, KB, TC], bf16, tag="xres")
            px1_t = ab_pool.tile([P, JA, 2, PX1], f8, tag="pax1")
            px2_t = ab_pool.tile([P, JA, 2, 512], f8, tag="pax2")
            b_t = ab_pool.tile([P, MB, P], bf16, tag="b")
            gate_t = ab_pool.tile([P, TC], f32, tag="gate")

            # Warmup is DMA-bound: ~14MB (w0/w1, x, pax, gate) streams on two
            # rings while the PE has only Phase A + mb0 + mb1 (~51us) to chew.
            # Model each ring's arrival times and emit DMAs/PE work in a
            # greedy arrival-matched order.
            BASE, BW = 4500.0, 0.117  # ring startup ns, bytes/ns per ring
            t_sc, t_sy = BASE, BASE
            rdy = {}

            def arr_sy(key, nbytes):
                nonlocal t_sy
                t_sy += nbytes / BW
                rdy[key] = t_sy

            def arr_sc(key, nbytes):
                nonlocal t_sc
                t_sc += nbytes / BW
                rdy[key] = t_sc

            # ---- sync ring: x k0-1, then 1-ktile bites, gate, b (split)
            XKB = 128 * TC * 2  # bytes per x k-tile
            nc.sync.dma_start(out=x_res[:, 0:2, :], in_=x_d[:, 0:2, :])
            arr_sy(("x", 0), 2 * XKB)
            rdy[("x", 1)] = rdy[("x", 0)]
            for k in range(2, 18):
                nc.sync.dma_start(out=x_res[:, k:k + 1, :], in_=x_d[:, k:k + 1, :])
                arr_sy(("x", k), XKB)
            nc.sync.dma_start(out=gate_t[:], in_=g_d[:])
            arr_sy("gate", TC * 4 * 128)
            for (lo, hi) in [(18, 21), (21, KB)]:
                nc.sync.dma_start(out=x_res[:, lo:hi, :], in_=x_d[:, lo:hi, :])
                arr_sy(("x", lo), (hi - lo) * XKB)
                for k in range(lo, hi):
                    rdy[("x", k)] = rdy[("x", lo)]
            nc.sync.dma_start(out=b_t[:, 0:8], in_=b_d[:, 0:8])
            nc.sync.dma_start(out=b_t[:, 8:MB], in_=b_d[:, 8:MB])

            def load_w(mb, queue):
                w_t = w_pool.tile([P, KB, P], bf16, tag="w", name=f"w{mb}")
                queue.dma_start(out=w_t[:], in_=w_d[mb])
                w8_t = w_pool.tile([P, NP8, 2, P], f8, tag="w8", name=f"w8{mb}")
                queue.dma_start(out=w8_t[:], in_=w8_d[mb])
                return w_t, w8_t

            # ---- scalar ring: w0/w1 pieces, pax1/pax2 chunks interleaved
            WKB = 128 * P * 2       # bytes per w k-tile
            PX1B = 128 * 2 * PX1    # bytes per pax1 j
            PX2B = 128 * 2 * 512    # bytes per pax2 j
            W8B = 128 * NP8 * 2 * P

            w0_t = w_pool.tile([P, KB, P], bf16, tag="w", name="w0")
            w1_t = w_pool.tile([P, KB, P], bf16, tag="w", name="w1")
            w80_t = w_pool.tile([P, NP8, 2, P], f8, tag="w8", name="w80")
            w81_t = w_pool.tile([P, NP8, 2, P], f8, tag="w8", name="w81")

            def w_piece(w_t, wmb, lo, hi, key):
                nc.scalar.dma_start(out=w_t[:, lo:hi, :], in_=w_d[wmb, :, lo:hi, :])
                arr_sc((key, lo), (hi - lo) * WKB)
                for k in range(lo, hi):
                    rdy[(key, k)] = rdy[(key, lo)]

            def px1_piece(lo, hi):
                nc.scalar.dma_start(out=px1_t[:, lo:hi], in_=px1_d[:, lo:hi])
                arr_sc(("p1", lo), (hi - lo) * PX1B)
                for j in range(lo, hi):
                    rdy[("p1", j)] = rdy[("p1", lo)]

            def px2_piece(lo, hi):
                nc.scalar.dma_start(out=px2_t[:, lo:hi], in_=px2_d[:, lo:hi])
                arr_sc(("p2", lo), (hi - lo) * PX2B)
                for j in range(lo, hi):
                    rdy[("p2", j)] = rdy[("p2", lo)]

            w_piece(w0_t, 0, 0, 4, "w0")
            px1_piece(0, 1)
            px1_piece(1, 2)
            px1_piece(2, 4)
            w_piece(w0_t, 0, 4, 12, "w0")
            px1_piece(4, 6)
            px1_piece(6, 8)
            w_piece(w1_t, 1, 0, 6, "w1")
            w_piece(w0_t, 0, 12, KB, "w0")
            nc.scalar.dma_start(out=w80_t[:], in_=w8_d[0])
            arr_sc("w80", W8B)
            px1_piece(8, 10)
            px1_piece(10, 12)
            w_piece(w1_t, 1, 6, 12, "w1")
            px1_piece(12, 14)
            px1_piece(14, JA)
            w_piece(w1_t, 1, 12, KB, "w1")
            nc.scalar.dma_start(out=w81_t[:], in_=w8_d[1])
            arr_sc("w81", W8B)
            px2_piece(0, 4)
            px2_piece(4, 8)
            px2_piece(8, 12)
            px2_piece(12, JA)

            # ---------------- PE emission ------------------------------------
            # Phase A current-half psums (3 banks) + mb0/mb1 (4) = 7 of 8.
            ps_a = [ps_pool.tile([P, 512], f32, tag="ps", name=f"ps_a{i}")
                    for i in range(3)]
            ps0 = [ps_pool.tile([P, 512], f32, tag="ps", name=f"pm0_{h}")
                   for h in range(NH)]
            ps1 = [ps_pool.tile([P, 512], f32, tag="ps", name=f"pm1_{h}")
                   for h in range(NH)]

            def a_jgroup(ps3, jlo, jhi, h):
                mv = (lambda j: px1_t[:, j, :, 3 * P:]) if h == 0 else \
                     (lambda j: px2_t[:, j, :, :])
                for j in range(jlo, jhi):
                    for i in range(3):
                        nc.tensor.matmul(
                            ps3[i][:],
                            px1_t[:, j, :, i * P:(i + 1) * P],
                            mv(j),
                            start=(j == 0), stop=(j == JA - 1),
                            perf_mode=DR,
                        )

            def mb_ksub(pss, w_t, klo, khi):
                for k in range(klo, khi):
                    for h in range(NH):
                        nc.tensor.matmul(
                            pss[h][:],
                            w_t[:, k, :],
                            x_res[:, k, h * 512:(h + 1) * 512],
                            start=(k == 0), stop=False,
                        )

            def mb_fp8(pss, w8_t):
                for jp in range(NP8):
                    j = KB // 2 + jp
                    for h in range(NH):
                        mv = px1_t[:, j, :, 3 * P:] if h == 0 else px2_t[:, j, :, :]
                        nc.tensor.matmul(
                            pss[h][:],
                            w8_t[:, jp, :, :],
                            mv,
                            start=False, stop=False,
                            perf_mode=DR,
                        )

            ag = [ag_pool.tile([P, TC], bf16, tag=f"ag{i}", name=f"ag{i}")
                  for i in range(3)]

            def gate_half(ps3, h):
                sl = slice(h * 512, (h + 1) * 512)
                for i in range(3):
                    nc.vector.tensor_mul(ag[i][:, sl], ps3[i][:], gate_t[:, sl])

            # Greedy warmup: emit the PE work unit whose operands arrive
            # earliest, tracking modeled PE time. Units: A0[j]/A1[j] (3 DR
            # mms each), M0[k]/M1[k] (2 bf16 mms each). A1 only after gate0
            # (psum bank reuse).
            MM_BF, MM_DR = 440.0, 715.0  # modeled 2x bf16 / 3x DR unit ns
            ps_a1 = [None, None, None]
            streams = {
                "A0": list(range(JA)), "A1": list(range(JA)),
                "M0": list(range(KB)), "M1": list(range(KB)),
            }
            gate0_done = False
            pe_t = BASE

            def unit_ready(s):
                if not streams[s]:
                    return None
                h = streams[s][0]
                if s == "A0":
                    return rdy[("p1", h)]
                if s == "A1":
                    if not gate0_done:
                        return None
                    return rdy[("p2", h)]
                if s == "M0":
                    return max(rdy[("w0", h)], rdy[("x", h)])
                return max(rdy[("w1", h)], rdy[("x", h)])

            while any(streams.values()):
                cand = [(unit_ready(s), s) for s in streams if unit_ready(s) is not None]
                t_r, s = min(cand)
                u = streams[s].pop(0)
                pe_t = max(pe_t, t_r)
                if s == "A0":
                    a_jgroup(ps_a, u, u + 1, 0)
                    pe_t += MM_DR
                    if u == JA - 1:
                        gate_half(ps_a, 0)
                        gate0_done = True
                        ps_a1[:] = [
                            ps_pool.tile([P, 512], f32, tag="ps", name=f"ps_b{i}")
                            for i in range(3)]
                elif s == "A1":
                    a_jgroup(ps_a1, u, u + 1, 1)
                    pe_t += MM_DR
                    if u == JA - 1:
                        gate_half(ps_a1, 1)
                elif s == "M0":
                    mb_ksub(ps0, w0_t, u, u + 1)
                    pe_t += MM_BF
                else:
                    mb_ksub(ps1, w1_t, u, u + 1)
                    pe_t += MM_BF

            mb_fp8(ps0, w80_t)
            mb_fp8(ps1, w81_t)

            def finish_mb(mb, pss, h_list=None):
                i = 0 if mb < Q_SIZE // P else (1 if mb < (Q_SIZE + KV_SIZE) // P else 2)
                st = st_pool.tile([P, TC], f32, tag="st", name=f"st{mb}")
                oq = nc.sync if mb % 2 == 0 else nc.scalar
                for h in (h_list if h_list is not None else range(NH)):
                    nc.tensor.matmul(
                        pss[h][:],
                        b_t[:, mb, :],
                        ag[i][:, h * 512:(h + 1) * 512],
                        start=False, stop=True,
                    )
                    nc.vector.tensor_scalar_mul(
                        st[:, h * 512:(h + 1) * 512], pss[h][:],
                        float(1.0 / SP))
                    oq.dma_start(
                        out=y_d[mb, :, h * 512:(h + 1) * 512],
                        in_=st[:, h * 512:(h + 1) * 512],
                    )
                return st

            finish_mb(0, ps0)
            finish_mb(1, ps1)

            # ------------- Phase C: remaining mb chains ---------------------
            # h-sequential: one psum bank per half, half0's B/copy/DMA overlap
            # half1's chain; last mb's final DMAs split to shorten the tail.
            for mb in range(2, MB):
                w_t, w8_t = load_w(mb, nc.scalar)
                i = 0 if mb < Q_SIZE // P else (1 if mb < (Q_SIZE + KV_SIZE) // P else 2)
                st = st_pool.tile([P, TC], f32, tag="st", name=f"st{mb}")
                oq = nc.sync if mb % 2 == 0 else nc.scalar
                pieces = 2 if mb >= MB - 2 else 1
                for h in range(NH):
                    ps = ps_pool.tile([P, 512], f32, tag="ps", name=f"pm{mb}_{h}")
                    for k in range(KB):
                        nc.tensor.matmul(
                            ps[:], w_t[:, k, :],
                            x_res[:, k, h * 512:(h + 1) * 512],
                            start=(k == 0), stop=False,
                        )
                    for jp in range(NP8):
                        j = KB // 2 + jp
                        mv = px1_t[:, j, :, 3 * P:] if h == 0 else px2_t[:, j, :, :]
                        nc.tensor.matmul(
                            ps[:], w8_t[:, jp, :, :], mv,
                            start=False, stop=False, perf_mode=DR,
                        )
                    nc.tensor.matmul(
                        ps[:], b_t[:, mb, :],
                        ag[i][:, h * 512:(h + 1) * 512],
                        start=False, stop=True,
                    )
                    nc.vector.tensor_scalar_mul(
                        st[:, h * 512:(h + 1) * 512], ps[:],
                        float(1.0 / SP))
                    pw = 512 // pieces
                    for pc in range(pieces):
                        lo = h * 512 + pc * pw
                        oq.dma_start(
                            out=y_d[mb, :, lo:lo + pw],
                            in_=st[:, lo:lo + pw],
                        )

    nc.compile()
    return nc


def _get_nc():
    if "nc" not in _CACHE:
        _CACHE["nc"] = _build_nc()
    return _CACHE["nc"]


def _prep_in_maps(x, W, lora_A, lora_B_q, lora_B_k, lora_B_v, scaling, token_to_slot):
    f = np.float32
    bf = ml_dtypes.bfloat16
    x = np.ascontiguousarray(x, dtype=f)
    W = np.ascontiguousarray(W, dtype=f)

    # x shard, moving operand: [c, p, ka, tl]  (h = ka*128 + p, t = c*1024 + tl)
    x_f32 = np.ascontiguousarray(
        x.reshape(NCORES, TC, KA, P).transpose(0, 3, 2, 1))
    x_sh = np.ascontiguousarray(x_f32[:, :, :KB, :]).astype(bf)
    # W stationary: [mb, p, ka, dl]  (d = mb*128 + dl)  -- replicated.
    # bf16 part pre-scaled by SP to match the fp8 psum scale; the last
    # 2*NP8 k-tiles go as fp8(W*1024) DoubleRow pairs.
    w_all = W.reshape(MB, P, KA, P).transpose(0, 3, 2, 1)
    w_t = np.ascontiguousarray(w_all[:, :, :KB, :] * SP).astype(bf)
    # fp8 e4m3 copies for the LoRA down-proj (DoubleRow pairs of k-tiles);
    # pax1 = [A targets | x8 half0], pax2 = [x8 half1]. 1/(SX*SA) descale
    # folds into the gate below.
    SX, SA = np.float32(32.0), np.float32(1024.0)
    f8 = ml_dtypes.float8_e4m3
    w8 = np.ascontiguousarray(
        (w_all[:, :, KB:, :] * SA).astype(f8).reshape(MB, P, NP8, 2, P))
    x8 = (x_f32 * SX).astype(f8).reshape(NCORES, P, JA, 2, TC)
    a_f32 = np.ascontiguousarray(
        np.asarray(lora_A, dtype=f).reshape(S, 3, R, KA, P).transpose(4, 3, 1, 0, 2)
        .reshape(P, KA, 3, S * R))
    a8 = (a_f32 * SA).astype(f8).reshape(P, JA, 2, 3 * S * R)
    pax1 = np.concatenate(
        [np.broadcast_to(a8, (NCORES,) + a8.shape), x8[..., 0:512]], axis=-1)
    pax1 = np.ascontiguousarray(pax1)
    pax2 = np.ascontiguousarray(x8[..., 512:])
    # LoRA B stationary: [(s r), mb, dl] -- replicated
    bq = np.asarray(lora_B_q, dtype=f).transpose(0, 2, 1).reshape(S * R, Q_SIZE)
    bk = np.asarray(lora_B_k, dtype=f).transpose(0, 2, 1).reshape(S * R, KV_SIZE)
    bv = np.asarray(lora_B_v, dtype=f).transpose(0, 2, 1).reshape(S * R, KV_SIZE)
    b_t = np.ascontiguousarray(
        np.concatenate([bq, bk, bv], axis=1).reshape(S * R, MB, P)).astype(bf)
    # routing gate, expanded over ranks: [c, (s r), tl]. The LoRA psum is
    # already SP x true scale (x*32 times A*1024), which matches the main
    # psum scale, so the gate is just the per-slot scaling.
    slot = np.asarray(token_to_slot).reshape(NCORES, TC)
    g = (slot[:, None, :] == np.arange(S, dtype=slot.dtype)[None, :, None])
    g = g.astype(f) * np.asarray(scaling, dtype=f)[None, :, None]
    gate = np.ascontiguousarray(np.repeat(g, R, axis=1))

    in_maps = []
    for c in range(NCORES):
        in_maps.append({
            "x_sh": x_sh[c],
            "w_t": w_t,
            "w8_t": w8,
            "pax1": pax1[c],
            "pax2": pax2[c],
            "b_t": b_t,
            "gate": gate[c],
        })
    return in_maps


def _assemble(results):
    out = np.empty((T, D), dtype=np.float32)
    for c in range(NCORES):
        out[c * TC:(c + 1) * TC, :] = results[c]["y_out"].reshape(D, TC).T
    return out


def _run(inputs, trace=False):
    from concourse.bass_utils import run_bass_kernel_spmd
    nc = _get_nc()
    in_maps = _prep_in_maps(**inputs)
    res = run_bass_kernel_spmd(
        nc, in_maps, core_ids=list(range(NCORES)), trace=trace)
    return res


def kernel(**inputs) -> np.ndarray:
    res = _run(inputs, trace=False)
    return _assemble(res.results)


if __name__ == "__main__":
    rng = np.random.default_rng(0)
    ins = {
        "x": rng.standard_normal((T, HID)).astype(np.float32),
        "W": (rng.standard_normal((D, HID)) * 0.02).astype(np.float32),
        "lora_A": (rng.standard_normal((S, 3, R, HID)) * 0.02).astype(np.float32),
        "lora_B_q": (rng.standard_normal((S, Q_SIZE, R)) * 0.02).astype(np.float32),
        "lora_B_k": (rng.standard_normal((S, KV_SIZE, R)) * 0.02).astype(np.float32),
        "lora_B_v": (rng.standard_normal((S, KV_SIZE, R)) * 0.02).astype(np.float32),
        "scaling": rng.uniform(0.5, 2.0, S).astype(np.float32),
        "token_to_slot": rng.integers(0, S, T).astype(np.int32),
    }
    out = kernel(**ins)
    print("out", out.shape, out.dtype)


# revision 17
# speedup vs baseline: 1.0296x; 1.0024x over previous
"""LoRA QKV fused projection kernel for 8 TRN2 NeuronCores.

Reference computation (T=8192 tokens, HID=4096, D=6144 out, S=8 slots, R=16):
    y = x @ W.T
    a[t,s,i,r] = sum_h x[t,h] * lora_A[s,i,r,h]         (down-proj, all slots)
    a *= onehot(token_to_slot)[t,s] * scaling[s]         (routing gate)
    d[t, :] = concat_i( sum_{s,r} a[t,s,i,r] * B_i[s,:,r] )   (up-proj)
    out = y + d
Sharding: data-parallel over tokens; core c owns tokens [c*1024, (c+1)*1024).

Per-core dataflow (v5):
  * Phase A (LoRA down-proj aT = A @ x) in fp8 e4m3 DoubleRow over all 16
    k-tile PAIRS, one token half at a time (3 psum banks), j-major, paced
    by the pax tables streaming on the scalar ring. pax is split into
    pax1 = [A targets | x8 half0] and pax2 = [x8 half1] so half0 can
    start early.
  * mb0 and mb1's bf16 k-chains interleave BETWEEN Phase A j-groups
    (banks: 3 A + 2+2 = 7 of 8); their fp8 pairs + B-matmuls run right
    after the gate. PE stays fed from ~5us while ~14MB of warmup DMA
    (w0/w1, x, pax, gate) streams on two rings (gpsimd as a third ring
    measured WORSE: its DRAIN ops add ~14us of idle).
  * Phase B: routing gate (host-built onehot*scaling, expanded over rank)
    applied on DVE per half: ag = psum_a * gate, written bf16.
  * Phase C per output row-block mb (48): 24 bf16 k-tiles (W pre-scaled by
    SP=2^15 to match the fp8 psum scale) + 4 fp8 DoubleRow k-tile PAIRS
    accumulate into 2 psum tiles (token halves); the LoRA up-proj
    B[mb] @ ag[i] accumulates INTO THE SAME psum (start=False), fusing
    base + delta. psum->sbuf copy descales by 1/SP exactly, then one DMA
    out per half, alternating sync/scalar rings. The last two mbs run
    h-sequentially so half0's copy+DMA overlap half1's chain (shorter
    tail).

Precision (gate 2e-2): bf16 main + fp8 on 8/32 k-tiles + fp8 LoRA delta
measured 1.921e-2 on the harness inputs (numpy sim matches HW to ~1e-6).
fp8 e4m3 DoubleRow measured ~234ns per 512-col pair vs 2x218ns bf16.
"""

import numpy as np
import ml_dtypes

# problem shape (hardcoded per harness contract)
T = 8192
HID = 4096
Q_SIZE = 4096
KV_SIZE = 1024
D = Q_SIZE + 2 * KV_SIZE  # 6144
S = 8
R = 16
NCORES = 8
P = 128

TC = T // NCORES          # 1024 tokens per core
MB = D // P               # 48 output row-blocks of 128
KA = HID // P             # 32 k-tiles
NP8 = 4                   # k-tile PAIRS of the main GEMM done in fp8 DoubleRow
KB = KA - 2 * NP8         # 24 k-tiles of the main GEMM done in bf16
NH = TC // 512            # 2 token halves (psum bank = 512 fp32)
JA = KA // 2              # 16 k-tile pairs
SP = np.float32(32768.0)  # psum scale SX*SA: bf16 W is pre-scaled by SP so
                          # fp8 (x*32)(W*1024) products accumulate coherently;
                          # the final psum->sbuf copy descales by 1/SP (exact)

_CACHE = {}


def _build_nc():
    import concourse.mybir as mybir
    import concourse.tile as tile
    from concourse import bacc

    bf16 = mybir.dt.bfloat16
    f32 = mybir.dt.float32
    f8 = mybir.dt.float8e4
    DR = mybir.MatmulPerfMode.DoubleRow

    nc = bacc.Bacc(None, target_bir_lowering=False, debug=False)

    # ---- DRAM parameters (per-core shapes)
    PX1 = 3 * P + 512  # pax1 row: 3 A targets then x8 half0
    x_d = nc.declare_dram_parameter("x_sh", [P, KB, TC], bf16, isOutput=False)
    w_d = nc.declare_dram_parameter("w_t", [MB, P, KB, P], bf16, isOutput=False)
    w8_d = nc.declare_dram_parameter("w8_t", [MB, P, NP8, 2, P], f8, isOutput=False)
    px1_d = nc.declare_dram_parameter("pax1", [P, JA, 2, PX1], f8, isOutput=False)
    px2_d = nc.declare_dram_parameter("pax2", [P, JA, 2, 512], f8, isOutput=False)
    b_d = nc.declare_dram_parameter("b_t", [P, MB, P], bf16, isOutput=False)
    g_d = nc.declare_dram_parameter("gate", [P, TC], f32, isOutput=False)
    y_d = nc.declare_dram_parameter("y_out", [MB, P, TC], f32, isOutput=True)

    with tile.TileContext(nc) as tc:
        with tc.tile_pool(name="xres", bufs=1) as xres_pool, \
             tc.tile_pool(name="wp", bufs=3) as w_pool, \
             tc.tile_pool(name="ab", bufs=1) as ab_pool, \
             tc.tile_pool(name="agp", bufs=1) as ag_pool, \
             tc.tile_pool(name="stp", bufs=3) as st_pool, \
             tc.tile_pool(name="psum", bufs=8, space="PSUM") as ps_pool:

            # resident operands
            x_res = xres_pool.tile([P, KB, TC], bf16, tag="xres")
            px1_t = ab_pool.tile([P, JA, 2, PX1], f8, tag="pax1")
            px2_t = ab_pool.tile([P, JA, 2, 512], f8, tag="pax2")
            b_t = ab_pool.tile([P, MB, P], bf16, tag="b")
            gate_t = ab_pool.tile([P, TC], f32, tag="gate")

            # Warmup is DMA-bound: ~14MB (w0/w1, x, pax, gate) streams on two
            # rings while the PE has only Phase A + mb0 + mb1 (~51us) to chew.
            # Model each ring's arrival times and emit DMAs/PE work in a
            # greedy arrival-matched order.
            BASE, BW = 4500.0, 0.117  # ring startup ns, bytes/ns per ring
            t_sc, t_sy = BASE, BASE
            rdy = {}

            def arr_sy(key, nbytes):
                nonlocal t_sy
                t_sy += nbytes / BW
                rdy[key] = t_sy

            def arr_sc(key, nbytes):
                nonlocal t_sc
                t_sc += nbytes / BW
                rdy[key] = t_sc

            # ---- sync ring: 1-ktile x bites, gate early, b (split)
            XKB = 128 * TC * 2  # bytes per x k-tile
            for k in range(0, 10):
                nc.sync.dma_start(out=x_res[:, k:k + 1, :], in_=x_d[:, k:k + 1, :])
                arr_sy(("x", k), XKB)
            nc.sync.dma_start(out=gate_t[:], in_=g_d[:])
            arr_sy("gate", TC * 4 * 128)
            for k in range(10, 18):
                nc.sync.dma_start(out=x_res[:, k:k + 1, :], in_=x_d[:, k:k + 1, :])
                arr_sy(("x", k), XKB)
            for (lo, hi) in [(18, 21), (21, KB)]:
                nc.sync.dma_start(out=x_res[:, lo:hi, :], in_=x_d[:, lo:hi, :])
                arr_sy(("x", lo), (hi - lo) * XKB)
                for k in range(lo, hi):
                    rdy[("x", k)] = rdy[("x", lo)]
            nc.sync.dma_start(out=b_t[:, 0:8], in_=b_d[:, 0:8])
            nc.sync.dma_start(out=b_t[:, 8:MB], in_=b_d[:, 8:MB])

            def load_w(mb, queue):
                w_t = w_pool.tile([P, KB, P], bf16, tag="w", name=f"w{mb}")
                queue.dma_start(out=w_t[:], in_=w_d[mb])
                w8_t = w_pool.tile([P, NP8, 2, P], f8, tag="w8", name=f"w8{mb}")
                queue.dma_start(out=w8_t[:], in_=w8_d[mb])
                return w_t, w8_t

            # ---- scalar ring: w0/w1 pieces, pax1/pax2 chunks interleaved
            WKB = 128 * P * 2       # bytes per w k-tile
            PX1B = 128 * 2 * PX1    # bytes per pax1 j
            PX2B = 128 * 2 * 512    # bytes per pax2 j
            W8B = 128 * NP8 * 2 * P

            w0_t = w_pool.tile([P, KB, P], bf16, tag="w", name="w0")
            w1_t = w_pool.tile([P, KB, P], bf16, tag="w", name="w1")
            w80_t = w_pool.tile([P, NP8, 2, P], f8, tag="w8", name="w80")
            w81_t = w_pool.tile([P, NP8, 2, P], f8, tag="w8", name="w81")

            def w_piece(w_t, wmb, lo, hi, key):
                nc.scalar.dma_start(out=w_t[:, lo:hi, :], in_=w_d[wmb, :, lo:hi, :])
                arr_sc((key, lo), (hi - lo) * WKB)
                for k in range(lo, hi):
                    rdy[(key, k)] = rdy[(key, lo)]

            def px1_piece(lo, hi):
                nc.scalar.dma_start(out=px1_t[:, lo:hi], in_=px1_d[:, lo:hi])
                arr_sc(("p1", lo), (hi - lo) * PX1B)
                for j in range(lo, hi):
                    rdy[("p1", j)] = rdy[("p1", lo)]

            def px2_piece(lo, hi):
                nc.scalar.dma_start(out=px2_t[:, lo:hi], in_=px2_d[:, lo:hi])
                arr_sc(("p2", lo), (hi - lo) * PX2B)
                for j in range(lo, hi):
                    rdy[("p2", j)] = rdy[("p2", lo)]

            w_piece(w0_t, 0, 0, 2, "w0")
            px1_piece(0, 1)
            px1_piece(1, 2)
            w_piece(w0_t, 0, 2, 8, "w0")
            px1_piece(2, 4)
            px1_piece(4, 6)
            w_piece(w0_t, 0, 8, 16, "w0")
            px1_piece(6, 8)
            w_piece(w1_t, 1, 0, 6, "w1")
            w_piece(w0_t, 0, 16, KB, "w0")
            nc.scalar.dma_start(out=w80_t[:], in_=w8_d[0])
            arr_sc("w80", W8B)
            px1_piece(8, 10)
            px1_piece(10, 12)
            w_piece(w1_t, 1, 6, 12, "w1")
            px1_piece(12, 14)
            px1_piece(14, JA)
            px2_piece(0, 4)
            w_piece(w1_t, 1, 12, KB, "w1")
            nc.scalar.dma_start(out=w81_t[:], in_=w8_d[1])
            arr_sc("w81", W8B)
            px2_piece(4, 8)
            px2_piece(8, 12)
            px2_piece(12, JA)

            # ---------------- PE emission ------------------------------------
            # Phase A current-half psums (3 banks) + mb0/mb1 (4) = 7 of 8.
            ps_a = [ps_pool.tile([P, 512], f32, tag="ps", name=f"ps_a{i}")
                    for i in range(3)]
            ps0 = [ps_pool.tile([P, 512], f32, tag="ps", name=f"pm0_{h}")
                   for h in range(NH)]
            ps1 = [ps_pool.tile([P, 512], f32, tag="ps", name=f"pm1_{h}")
                   for h in range(NH)]

            def a_jgroup(ps3, jlo, jhi, h):
                mv = (lambda j: px1_t[:, j, :, 3 * P:]) if h == 0 else \
                     (lambda j: px2_t[:, j, :, :])
                for j in range(jlo, jhi):
                    for i in range(3):
                        nc.tensor.matmul(
                            ps3[i][:],
                            px1_t[:, j, :, i * P:(i + 1) * P],
                            mv(j),
                            start=(j == 0), stop=(j == JA - 1),
                            perf_mode=DR,
                        )

            def mb_ksub(pss, w_t, klo, khi):
                for k in range(klo, khi):
                    for h in range(NH):
                        nc.tensor.matmul(
                            pss[h][:],
                            w_t[:, k, :],
                            x_res[:, k, h * 512:(h + 1) * 512],
                            start=(k == 0), stop=False,
                        )

            def mb_fp8(pss, w8_t):
                for jp in range(NP8):
                    j = KB // 2 + jp
                    for h in range(NH):
                        mv = px1_t[:, j, :, 3 * P:] if h == 0 else px2_t[:, j, :, :]
                        nc.tensor.matmul(
                            pss[h][:],
                            w8_t[:, jp, :, :],
                            mv,
                            start=False, stop=False,
                            perf_mode=DR,
                        )

            ag = [ag_pool.tile([P, TC], bf16, tag=f"ag{i}", name=f"ag{i}")
                  for i in range(3)]

            def gate_half(ps3, h):
                sl = slice(h * 512, (h + 1) * 512)
                for i in range(3):
                    nc.vector.tensor_mul(ag[i][:, sl], ps3[i][:], gate_t[:, sl])

            # Greedy warmup: emit the PE work unit whose operands arrive
            # earliest, tracking modeled PE time. Units: A0[j]/A1[j] (3 DR
            # mms each), M0[k]/M1[k] (2 bf16 mms each). A1 only after gate0
            # (psum bank reuse).
            MM_BF, MM_DR = 440.0, 715.0  # modeled 2x bf16 / 3x DR unit ns
            ps_a1 = [None, None, None]
            streams = {
                "A0": list(range(JA)), "A1": list(range(JA)),
                "M0": list(range(KB)), "M1": list(range(KB)),
            }
            gate0_done = False
            pe_t = BASE

            def unit_ready(s):
                if not streams[s]:
                    return None
                h = streams[s][0]
                if s == "A0":
                    return rdy[("p1", h)]
                if s == "A1":
                    if not gate0_done:
                        return None
                    return rdy[("p2", h)]
                if s == "M0":
                    return max(rdy[("w0", h)], rdy[("x", h)])
                return max(rdy[("w1", h)], rdy[("x", h)])

            while any(streams.values()):
                cand = [(unit_ready(s), s) for s in streams if unit_ready(s) is not None]
                t_r, s = min(cand)
                u = streams[s].pop(0)
                pe_t = max(pe_t, t_r)
                if s == "A0":
                    a_jgroup(ps_a, u, u + 1, 0)
                    pe_t += MM_DR
                    if u == JA - 1:
                        gate_half(ps_a, 0)
                        gate0_done = True
                        ps_a1[:] = [
                            ps_pool.tile([P, 512], f32, tag="ps", name=f"ps_b{i}")
                            for i in range(3)]
                elif s == "A1":
                    a_jgroup(ps_a1, u, u + 1, 1)
                    pe_t += MM_DR
                    if u == JA - 1:
                        gate_half(ps_a1, 1)
                elif s == "M0":
                    mb_ksub(ps0, w0_t, u, u + 1)
                    pe_t += MM_BF
                else:
                    mb_ksub(ps1, w1_t, u, u + 1)
                    pe_t += MM_BF

            mb_fp8(ps0, w80_t)
            mb_fp8(ps1, w81_t)

            def finish_mb(mb, pss, h_list=None):
                i = 0 if mb < Q_SIZE // P else (1 if mb < (Q_SIZE + KV_SIZE) // P else 2)
                st = st_pool.tile([P, TC], f32, tag="st", name=f"st{mb}")
                oq = nc.sync if mb % 2 == 0 else nc.scalar
                for h in (h_list if h_list is not None else range(NH)):
                    nc.tensor.matmul(
                        pss[h][:],
                        b_t[:, mb, :],
                        ag[i][:, h * 512:(h + 1) * 512],
                        start=False, stop=True,
                    )
                    nc.vector.tensor_scalar_mul(
                        st[:, h * 512:(h + 1) * 512], pss[h][:],
                        float(1.0 / SP))
                    oq.dma_start(
                        out=y_d[mb, :, h * 512:(h + 1) * 512],
                        in_=st[:, h * 512:(h + 1) * 512],
                    )
                return st

            finish_mb(0, ps0)
            finish_mb(1, ps1)

            # ------------- Phase C: remaining mb chains ---------------------
            # h-sequential: one psum bank per half, half0's B/copy/DMA overlap
            # half1's chain; last mb's final DMAs split to shorten the tail.
            for mb in range(2, MB):
                w_t, w8_t = load_w(mb, nc.scalar)
                i = 0 if mb < Q_SIZE // P else (1 if mb < (Q_SIZE + KV_SIZE) // P else 2)
                st = st_pool.tile([P, TC], f32, tag="st", name=f"st{mb}")
                oq = nc.sync if mb % 2 == 0 else nc.scalar
                pieces = 2 if mb >= MB - 2 else 1
                for h in range(NH):
                    ps = ps_pool.tile([P, 512], f32, tag="ps", name=f"pm{mb}_{h}")
                    for k in range(KB):
                        nc.tensor.matmul(
                            ps[:], w_t[:, k, :],
                            x_res[:, k, h * 512:(h + 1) * 512],
                            start=(k == 0), stop=False,
                        )
                    for jp in range(NP8):
                        j = KB // 2 + jp
                        mv = px1_t[:, j, :, 3 * P:] if h == 0 else px2_t[:, j, :, :]
                        nc.tensor.matmul(
                            ps[:], w8_t[:, jp, :, :], mv,
                            start=False, stop=False, perf_mode=DR,
                        )
                    nc.tensor.matmul(
                        ps[:], b_t[:, mb, :],
                        ag[i][:, h * 512:(h + 1) * 512],
                        start=False, stop=True,
                    )
                    nc.vector.tensor_scalar_mul(
                        st[:, h * 512:(h + 1) * 512], ps[:],
                        float(1.0 / SP))
                    pw = 512 // pieces
                    for pc in range(pieces):
                        lo = h * 512 + pc * pw
                        # final mbs: spread pieces over both rings
                        q = (nc.sync, nc.scalar)[pc % 2] if pieces > 1 else oq
                        q.dma_start(
                            out=y_d[mb, :, lo:lo + pw],
                            in_=st[:, lo:lo + pw],
                        )

    nc.compile()
    return nc


def _get_nc():
    if "nc" not in _CACHE:
        _CACHE["nc"] = _build_nc()
    return _CACHE["nc"]


def _prep_in_maps(x, W, lora_A, lora_B_q, lora_B_k, lora_B_v, scaling, token_to_slot):
    f = np.float32
    bf = ml_dtypes.bfloat16
    x = np.ascontiguousarray(x, dtype=f)
    W = np.ascontiguousarray(W, dtype=f)

    # x shard, moving operand: [c, p, ka, tl]  (h = ka*128 + p, t = c*1024 + tl)
    x_f32 = np.ascontiguousarray(
        x.reshape(NCORES, TC, KA, P).transpose(0, 3, 2, 1))
    x_sh = np.ascontiguousarray(x_f32[:, :, :KB, :]).astype(bf)
    # W stationary: [mb, p, ka, dl]  (d = mb*128 + dl)  -- replicated.
    # bf16 part pre-scaled by SP to match the fp8 psum scale; the last
    # 2*NP8 k-tiles go as fp8(W*1024) DoubleRow pairs.
    w_all = W.reshape(MB, P, KA, P).transpose(0, 3, 2, 1)
    w_t = np.ascontiguousarray(w_all[:, :, :KB, :] * SP).astype(bf)
    # fp8 e4m3 copies for the LoRA down-proj (DoubleRow pairs of k-tiles);
    # pax1 = [A targets | x8 half0], pax2 = [x8 half1]. 1/(SX*SA) descale
    # folds into the gate below.
    SX, SA = np.float32(32.0), np.float32(1024.0)
    f8 = ml_dtypes.float8_e4m3
    w8 = np.ascontiguousarray(
        (w_all[:, :, KB:, :] * SA).astype(f8).reshape(MB, P, NP8, 2, P))
    x8 = (x_f32 * SX).astype(f8).reshape(NCORES, P, JA, 2, TC)
    a_f32 = np.ascontiguousarray(
        np.asarray(lora_A, dtype=f).reshape(S, 3, R, KA, P).transpose(4, 3, 1, 0, 2)
        .reshape(P, KA, 3, S * R))
    a8 = (a_f32 * SA).astype(f8).reshape(P, JA, 2, 3 * S * R)
    pax1 = np.concatenate(
        [np.broadcast_to(a8, (NCORES,) + a8.shape), x8[..., 0:512]], axis=-1)
    pax1 = np.ascontiguousarray(pax1)
    pax2 = np.ascontiguousarray(x8[..., 512:])
    # LoRA B stationary: [(s r), mb, dl] -- replicated
    bq = np.asarray(lora_B_q, dtype=f).transpose(0, 2, 1).reshape(S * R, Q_SIZE)
    bk = np.asarray(lora_B_k, dtype=f).transpose(0, 2, 1).reshape(S * R, KV_SIZE)
    bv = np.asarray(lora_B_v, dtype=f).transpose(0, 2, 1).reshape(S * R, KV_SIZE)
    b_t = np.ascontiguousarray(
        np.concatenate([bq, bk, bv], axis=1).reshape(S * R, MB, P)).astype(bf)
    # routing gate, expanded over ranks: [c, (s r), tl]. The LoRA psum is
    # already SP x true scale (x*32 times A*1024), which matches the main
    # psum scale, so the gate is just the per-slot scaling.
    slot = np.asarray(token_to_slot).reshape(NCORES, TC)
    g = (slot[:, None, :] == np.arange(S, dtype=slot.dtype)[None, :, None])
    g = g.astype(f) * np.asarray(scaling, dtype=f)[None, :, None]
    gate = np.ascontiguousarray(np.repeat(g, R, axis=1))

    in_maps = []
    for c in range(NCORES):
        in_maps.append({
            "x_sh": x_sh[c],
            "w_t": w_t,
            "w8_t": w8,
            "pax1": pax1[c],
            "pax2": pax2[c],
            "b_t": b_t,
            "gate": gate[c],
        })
    return in_maps


def _assemble(results):
    out = np.empty((T, D), dtype=np.float32)
    for c in range(NCORES):
        out[c * TC:(c + 1) * TC, :] = results[c]["y_out"].reshape(D, TC).T
    return out


def _run(inputs, trace=False):
    from concourse.bass_utils import run_bass_kernel_spmd
    nc = _get_nc()
    in_maps = _prep_in_maps(**inputs)
    res = run_bass_kernel_spmd(
        nc, in_maps, core_ids=list(range(NCORES)), trace=trace)
    return res


def kernel(**inputs) -> np.ndarray:
    res = _run(inputs, trace=False)
    return _assemble(res.results)


if __name__ == "__main__":
    rng = np.random.default_rng(0)
    ins = {
        "x": rng.standard_normal((T, HID)).astype(np.float32),
        "W": (rng.standard_normal((D, HID)) * 0.02).astype(np.float32),
        "lora_A": (rng.standard_normal((S, 3, R, HID)) * 0.02).astype(np.float32),
        "lora_B_q": (rng.standard_normal((S, Q_SIZE, R)) * 0.02).astype(np.float32),
        "lora_B_k": (rng.standard_normal((S, KV_SIZE, R)) * 0.02).astype(np.float32),
        "lora_B_v": (rng.standard_normal((S, KV_SIZE, R)) * 0.02).astype(np.float32),
        "scaling": rng.uniform(0.5, 2.0, S).astype(np.float32),
        "token_to_slot": rng.integers(0, S, T).astype(np.int32),
    }
    out = kernel(**ins)
    print("out", out.shape, out.dtype)
